# revision 1
# baseline (speedup 1.0000x reference)
"""Trainium2 Bass kernel for nn_AdvancedLossFunction (8-core SPMD).

Sharding: 8 cores = 2 batches x 4 quarters. Core c handles batch b=c//4,
row/col quarter q=c%4 of the 4096x4096 point-cloud distance matrix, plus
1/8 of the elementwise (P=100000) losses.

Math (validated against the jax reference in a numpy prototype):
  chamfer = mean rowmin d + mean colmin d
  emd     = mean_b [mean_n A_n + mean_m C_m],  A_n = sum_m r*d (row softmax)
  gp      = mean |  ||g|| - 1 |,  g = d(emd)/d(pred) computed analytically:
      g_n = x_n * S_n - (H @ Y)_n,  H = G/d,
      G = 1/(B*N) r*(1-(d-A)/tau) + 1/(B*M) c*(1-(d-C)/tau)

Device strategy:
  - d^2 via PE matmuls (k=4 trick: lhsT=[-2x|1], rhs=[y|yy], + xx as ACT bias)
  - sqrt & 1/x via exp(+-0.5*ln(.)) so ALL transcendentals live in one ACT
    table set (natural_log_exp_and_others) -> zero table switches
  - row/col passes fp32; col stats AllGathered across the 4 cores of a batch
  - gradient pass in column orientation (m on partitions): coefficients are
    per-partition scalars, reductions via PE matmul with lhsT=[y0,y1,y2,1],
    elementwise H products in bf16 (gp term is error-tolerant: ||g||<<1)
  - each core outputs a [128,64] tile of partial sums; host combines in f64
"""
import numpy as np
from contextlib import ExitStack

import concourse.bass as bass
import concourse.bacc as bacc
import concourse.tile as tile
from concourse import mybir
from concourse.bass_utils import run_bass_kernel_spmd

F32 = mybir.dt.float32
BF16 = mybir.dt.bfloat16
AF = mybir.ActivationFunctionType
ALU = mybir.AluOpType

TAU = 0.02
INV_TAU = 50.0
B, P = 2, 100000
N = M = 4096
NQ = 1024          # quarter size
RS = NQ // 128     # 8 row stripes per quarter
GS = M // 128      # 32 grad stripes (all m)
EPS_D2 = 4e-5      # ln-domain clamp; folded into the xx/yy bias on host
EWP, EWF = 125, 200  # per-core elementwise slice 25000 = 125*200

FOCAL_GAMMA, FOCAL_ALPHA = 2.0, 0.75
W_SDF, W_OCC, W_NORMAL, W_CHAMFER, W_EMD, W_EMD_GP, W_UNC = \
    1.0, 1.0, 0.1, 1.0, 0.25, 0.05, 0.1

_CACHE = {}

import os
BISECT_NO_COLLECTIVE = bool(int(os.environ.get("K_NO_COLLECTIVE", "0")))
BISECT_NO_GRAD = bool(int(os.environ.get("K_NO_GRAD", "0")))
BISECT_NO_INPLACE = bool(int(os.environ.get("K_NO_INPLACE", "0")))
BISECT_NO_DUMPS = bool(int(os.environ.get("K_NO_DUMPS", "0")))
BISECT_NO_STATS = bool(int(os.environ.get("K_NO_STATS", "0")))
BISECT_NO_EW = bool(int(os.environ.get("K_NO_EW", "0")))


def _declare_inputs(nc):
    t = {}
    for name in ["sdfa", "sdfb", "uncw", "occp", "occt",
                 "nax", "nay", "naz", "nbx", "nby", "nbz"]:
        t[name] = nc.dram_tensor(name, [EWP, EWF], F32, kind="ExternalInput")
    t["XTq"] = nc.dram_tensor("XTq", [4, NQ], F32, kind="ExternalInput")
    t["YTm"] = nc.dram_tensor("YTm", [4, M], F32, kind="ExternalInput")
    t["YTnq"] = nc.dram_tensor("YTnq", [4, NQ], F32, kind="ExternalInput")
    t["YTn"] = nc.dram_tensor("YTn", [4, M], F32, kind="ExternalInput")
    t["XTm"] = nc.dram_tensor("XTm", [4, N], F32, kind="ExternalInput")
    t["XTg"] = nc.dram_tensor("XTg", [4, NQ], F32, kind="ExternalInput")
    t["xxqeT"] = nc.dram_tensor("xxqeT", [128, RS], F32, kind="ExternalInput")
    t["yyqeT"] = nc.dram_tensor("yyqeT", [128, RS], F32, kind="ExternalInput")
    t["yyfeT"] = nc.dram_tensor("yyfeT", [128, GS], F32, kind="ExternalInput")
    t["Y4g"] = nc.dram_tensor("Y4g", [128, 4 * GS], BF16, kind="ExternalInput")
    t["xq0"] = nc.dram_tensor("xq0", [128, RS], F32, kind="ExternalInput")
    t["xq1"] = nc.dram_tensor("xq1", [128, RS], F32, kind="ExternalInput")
    t["xq2"] = nc.dram_tensor("xq2", [128, RS], F32, kind="ExternalInput")
    return t


def _ew_stage(nc, tc, ctx, t, outsb, pool):
    """Elementwise losses on the core's 25000-point slice -> outsb cols 40-44."""
    def load(name):
        s = pool.tile([EWP, EWF], F32, tag="ewin_" + name)
        nc.sync.dma_start(s[:], t[name][:])
        return s

    sdfa, sdfb = load("sdfa"), load("sdfb")
    # L1: |a-b| summed
    diff = pool.tile([EWP, EWF], F32, tag="ewt0")
    nc.vector.tensor_tensor(out=diff[:], in0=sdfa[:], in1=sdfb[:], op=ALU.subtract)
    junk = pool.tile([EWP, EWF], F32, tag="ewt1")
    nc.scalar.activation(junk[:], diff[:], AF.Abs, accum_out=outsb[0:EWP, 40:41])

    occp, occt = load("occp"), load("occt")
    lnp = pool.tile([EWP, EWF], F32, tag="ewt2")
    nc.scalar.activation(lnp[:], occp[:], AF.Ln)
    j2 = pool.tile([EWP, EWF], F32, tag="ewt3")
    nc.vector.tensor_tensor(out=j2[:], in0=occt[:], in1=lnp[:], op=ALU.mult)
    jr = pool.tile([EWP, EWF], F32, tag="ewtr")
    nc.vector.tensor_scalar(out=jr[:], in0=j2[:], scalar1=1.0, scalar2=None,
                            op0=ALU.mult, op1=ALU.add, accum_out=outsb[0:EWP, 41:42])
    onemp = pool.tile([EWP, EWF], F32, tag="ewt4")
    nc.vector.tensor_scalar(out=onemp[:], in0=occp[:], scalar1=-1.0, scalar2=1.0,
                            op0=ALU.mult, op1=ALU.add)
    ln1mp = pool.tile([EWP, EWF], F32, tag="ewt5")
    nc.scalar.activation(ln1mp[:], onemp[:], AF.Ln)
    onemt = pool.tile([EWP, EWF], F32, tag="ewt6")
    nc.vector.tensor_scalar(out=onemt[:], in0=occt[:], scalar1=-1.0, scalar2=1.0,
                            op0=ALU.mult, op1=ALU.add)
    nc.vector.tensor_tensor(out=lnp[:], in0=onemt[:], in1=ln1mp[:], op=ALU.mult)
    nc.vector.tensor_scalar(out=jr[:], in0=lnp[:], scalar1=1.0, scalar2=None,
                            op0=ALU.mult, op1=ALU.add, accum_out=outsb[0:EWP, 42:43])

    # cosine similarity of normals
    nax, nay, naz = load("nax"), load("nay"), load("naz")
    nbx, nby, nbz = load("nbx"), load("nby"), load("nbz")
    dot = pool.tile([EWP, EWF], F32, tag="ewt0")
    nc.vector.tensor_tensor(out=dot[:], in0=nax[:], in1=nbx[:], op=ALU.mult)
    tmp = pool.tile([EWP, EWF], F32, tag="ewt1")
    nc.vector.tensor_tensor(out=tmp[:], in0=nay[:], in1=nby[:], op=ALU.mult)
    nc.vector.tensor_tensor(out=dot[:], in0=dot[:], in1=tmp[:], op=ALU.add)
    nc.vector.tensor_tensor(out=tmp[:], in0=naz[:], in1=nbz[:], op=ALU.mult)
    nc.vector.tensor_tensor(out=dot[:], in0=dot[:], in1=tmp[:], op=ALU.add)

    def norm2(cx, cy, cz, tag):
        n2 = pool.tile([EWP, EWF], F32, tag=tag)
        s1 = pool.tile([EWP, EWF], F32, tag=tag + "s")
        nc.scalar.activation(n2[:], cx[:], AF.Square)
        nc.scalar.activation(s1[:], cy[:], AF.Square)
        nc.vector.tensor_tensor(out=n2[:], in0=n2[:], in1=s1[:], op=ALU.add)
        nc.scalar.activation(s1[:], cz[:], AF.Square)
        nc.vector.tensor_tensor(out=n2[:], in0=n2[:], in1=s1[:], op=ALU.add)
        # 1/norm = exp(-0.5*ln(n2))
        nc.scalar.activation(s1[:], n2[:], AF.Ln)
        nc.scalar.activation(n2[:], s1[:], AF.Exp, scale=-0.5)
        return n2

    ra = norm2(nax, nay, naz, "ewt2")
    rb = norm2(nbx, nby, nbz, "ewt4")
    nc.vector.tensor_tensor(out=dot[:], in0=dot[:], in1=ra[:], op=ALU.mult)
    nc.vector.tensor_tensor(out=dot[:], in0=dot[:], in1=rb[:], op=ALU.mult)
    nc.vector.tensor_scalar(out=tmp[:], in0=dot[:], scalar1=1.0, scalar2=None,
                            op0=ALU.mult, op1=ALU.add, accum_out=outsb[0:EWP, 43:44])

    # uncertainty regularizer u*(1-u)
    uncw = load("uncw")
    onemu = pool.tile([EWP, EWF], F32, tag="ewt0")
    nc.vector.tensor_scalar(out=onemu[:], in0=uncw[:], scalar1=-1.0, scalar2=1.0,
                            op0=ALU.mult, op1=ALU.add)
    nc.vector.tensor_tensor(out=onemu[:], in0=uncw[:], in1=onemu[:], op=ALU.mult)
    nc.vector.tensor_scalar(out=tmp[:], in0=onemu[:], scalar1=1.0, scalar2=None,
                            op0=ALU.mult, op1=ALU.add, accum_out=outsb[0:EWP, 44:45])


def _stats_pass(nc, tc, ctx, lhsT_sb, rhs_sb, bias_sb, pools, outsb, outcol,
                minraw_t, mc_t, ssum, tsum):
    """One stats pass (row or col orientation): 8 stripes of [128, 4096].

    Computes per-stripe: raw min of (psum) [host adds xx for chamfer],
    clamped min mc, softmax sum s (-> ssum) and weighted sum t (-> tsum).
    """
    sbuf, psum = pools
    CH = 2048
    for i in range(RS):
        lhsT = lhsT_sb[:, i * 128:(i + 1) * 128]
        bias = bias_sb[:, i:i + 1]
        t1 = sbuf.tile([128, M], F32, tag="t1")
        for h in range(2):
            pd2 = psum.tile([128, CH], F32, tag="pd2")
            for j in range(4):
                nc.tensor.matmul(
                    pd2[:, j * 512:(j + 1) * 512], lhsT,
                    rhs_sb[:, h * CH + j * 512: h * CH + (j + 1) * 512],
                    start=True, stop=True)
            # exact reference clamp: d2 = max(psum + xx, 1e-12)  [PSUM -> SBUF]
            nc.vector.tensor_scalar(out=t1[:, h * CH:(h + 1) * CH], in0=pd2[:],
                                    scalar1=bias, scalar2=1e-12,
                                    op0=ALU.add, op1=ALU.max)
        # clamped min d2 (host takes sqrt in f64 for chamfer)
        nc.vector.tensor_reduce(out=minraw_t[:, i:i + 1], in_=t1[:],
                                axis=mybir.AxisListType.X, op=ALU.min)
        # mc = exp(0.5*ln(min d2)); bias50 = 50*mc
        lmin = sbuf.tile([128, 1], F32, tag="lmin")
        nc.scalar.activation(lmin[:], minraw_t[:, i:i + 1], AF.Ln)
        nc.scalar.activation(mc_t[:, i:i + 1], lmin[:], AF.Exp, scale=0.5)
        # t1 = ln(d2) (in place unless bisecting)
        if BISECT_NO_INPLACE:
            t1b = sbuf.tile([128, M], F32, tag="t1b")
            nc.scalar.activation(t1b[:], t1[:], AF.Ln)
            t1 = t1b
        else:
            nc.scalar.activation(t1[:], t1[:], AF.Ln)
        b50 = sbuf.tile([128, 1], F32, tag="b50")
        nc.scalar.mul(b50[:], mc_t[:, i:i + 1], INV_TAU)
        # d = exp(0.5 * t1)
        d = sbuf.tile([128, M], F32, tag="dstripe")
        nc.scalar.activation(d[:], t1[:], AF.Exp, scale=0.5)
        # e = exp(-50 d + 50 mc), s = sum e
        er = sbuf.tile([128, M], F32, tag="erstripe")
        nc.scalar.activation(er[:], d[:], AF.Exp, bias=b50[:], scale=-INV_TAU,
                             accum_out=ssum[:, i:i + 1])
        # t = sum e*d  (TT product + TS accumulate; TTR is broken on HW here)
        junk = sbuf.tile([128, M], F32, tag="junkstripe")
        nc.vector.tensor_tensor(out=junk[:], in0=er[:], in1=d[:], op=ALU.mult)
        nc.vector.tensor_scalar(out=er[:], in0=junk[:], scalar1=1.0,
                                scalar2=None, op0=ALU.mult, op1=ALU.add,
                                accum_out=tsum[:, i:i + 1])


def _build(trn_type="TRN2"):
    nc = bacc.Bacc(trn_type, target_bir_lowering=False)
    t = _declare_inputs(nc)
    out = nc.dram_tensor("out", [128, 64], F32, kind="ExternalOutput")

    with tile.TileContext(nc) as tc:
        with ExitStack() as ctx:
            persist = ctx.enter_context(tc.tile_pool(name="persist", bufs=1))
            dram = ctx.enter_context(tc.tile_pool(name="dram", bufs=1, space="DRAM"))

            outsb = persist.tile([128, 64], F32)
            nc.vector.memset(outsb[:], 0.0)

            # persistent stats tiles [128, RS] (stripe-major: n = i*128 + p)
            m1raw = persist.tile([128, RS], F32)   # raw row min(psum)
            m1c = persist.tile([128, RS], F32)     # clamped row min
            ssum = persist.tile([128, RS], F32)
            tsum = persist.tile([128, RS], F32)
            m2raw = persist.tile([128, RS], F32)
            m2c = persist.tile([128, RS], F32)
            usum = persist.tile([128, RS], F32)
            vsum = persist.tile([128, RS], F32)

            # ---- elementwise + stats passes ----
            if not BISECT_NO_EW:
                with ExitStack() as sctx:
                    ewpool = sctx.enter_context(tc.tile_pool(name="ew", bufs=1))
                    _ew_stage(nc, tc, sctx, t, outsb, ewpool)

            with ExitStack() as sctx:
                spool = sctx.enter_context(tc.tile_pool(name="sp", bufs=2))
                ppool = sctx.enter_context(
                    tc.tile_pool(name="pp", bufs=2, space="PSUM"))
                pts = sctx.enter_context(tc.tile_pool(name="pts", bufs=1))

                XTq = pts.tile([4, NQ], F32)
                nc.sync.dma_start(XTq[:], t["XTq"][:])
                YTm = pts.tile([4, M], F32)
                nc.sync.dma_start(YTm[:], t["YTm"][:])
                YTnq = pts.tile([4, NQ], F32)
                nc.sync.dma_start(YTnq[:], t["YTnq"][:])
                XTm = pts.tile([4, N], F32)
                nc.sync.dma_start(XTm[:], t["XTm"][:])
                xxqeT = pts.tile([128, RS], F32)
                nc.sync.dma_start(xxqeT[:], t["xxqeT"][:])
                yyqeT = pts.tile([128, RS], F32)
                nc.sync.dma_start(yyqeT[:], t["yyqeT"][:])

                # row pass: quarter rows x all m
                if not BISECT_NO_STATS:
                    _stats_pass(nc, tc, sctx, XTq, YTm, xxqeT, (spool, ppool),
                                outsb, 0, m1raw, m1c, ssum, tsum)
                    # col pass: quarter cols x all n
                    _stats_pass(nc, tc, sctx, YTnq, XTm, yyqeT, (spool, ppool),
                                outsb, 16, m2raw, m2c, usum, vsum)
                else:
                    for tt in (m1raw, m1c, ssum, tsum, m2raw, m2c, usum, vsum):
                        nc.vector.memset(tt[:], 1.0)

            # raw mins to output (host finishes chamfer in f64)
            nc.vector.tensor_copy(out=outsb[:, 0:RS], in_=m1raw[:])
            nc.vector.tensor_copy(out=outsb[:, 16:16 + RS], in_=m2raw[:])

            # ---- stats finalize: A, C, softmax coefficients ----
            fin = ctx.enter_context(tc.tile_pool(name="fin", bufs=1))
            sinv = fin.tile([128, RS], F32)
            nc.vector.reciprocal(out=sinv[:], in_=ssum[:])
            uinv = fin.tile([128, RS], F32)
            nc.vector.reciprocal(out=uinv[:], in_=usum[:])
            # A = t/s -> outsb[:, 8:16];  C = v/u -> outsb[:, 24:32]
            nc.vector.tensor_tensor(out=outsb[:, 8:16], in0=tsum[:], in1=sinv[:],
                                    op=ALU.mult)
            nc.vector.tensor_tensor(out=outsb[:, 24:32], in0=vsum[:], in1=uinv[:],
                                    op=ALU.mult)
            # w1r = (1+50A)/(B*N*s); w2r = 50/(B*N*s)   (bf16 for grad pass)
            scr = fin.tile([128, RS], F32)
            nc.vector.tensor_scalar(out=scr[:], in0=outsb[:, 8:16], scalar1=INV_TAU,
                                    scalar2=1.0, op0=ALU.mult, op1=ALU.add)
            w1rf = fin.tile([128, RS], F32)
            nc.vector.tensor_tensor(out=w1rf[:], in0=scr[:], in1=sinv[:], op=ALU.mult)
            w1r_bf = fin.tile([128, RS], BF16)
            nc.vector.tensor_scalar(out=w1r_bf[:], in0=w1rf[:],
                                    scalar1=1.0 / (B * N), scalar2=None, op0=ALU.mult)
            w2r_bf = fin.tile([128, RS], BF16)
            nc.vector.tensor_scalar(out=w2r_bf[:], in0=sinv[:],
                                    scalar1=INV_TAU / (B * N), scalar2=None,
                                    op0=ALU.mult)
            # col coefficients (f32, gathered below)
            nc.vector.tensor_scalar(out=scr[:], in0=outsb[:, 24:32], scalar1=INV_TAU,
                                    scalar2=1.0, op0=ALU.mult, op1=ALU.add)
            w1cf = fin.tile([128, RS], F32)
            nc.vector.tensor_tensor(out=w1cf[:], in0=scr[:], in1=uinv[:], op=ALU.mult)
            nc.vector.tensor_scalar(out=w1cf[:], in0=w1cf[:], scalar1=1.0 / (B * M),
                                    scalar2=None, op0=ALU.mult)
            w2cf = fin.tile([128, RS], F32)
            nc.vector.tensor_scalar(out=w2cf[:], in0=uinv[:],
                                    scalar1=INV_TAU / (B * M), scalar2=None,
                                    op0=ALU.mult)
            m2c50 = fin.tile([128, RS], F32)
            nc.scalar.mul(m2c50[:], m2c[:], INV_TAU)

            # ---- DRAM roundtrips: broadcasts + AllGather of col stats ----
            skip_dumps = BISECT_NO_DUMPS or BISECT_NO_STATS
            def dump_nmajor(sb_tile, dram_tile, off=0):
                if skip_dumps:
                    return
                # SBUF [128, RS] -> DRAM flat [NQ] with index i*128+p
                ap = bass.AP(tensor=dram_tile.tensor, offset=dram_tile.offset + off,
                             ap=[[1, 128], [128, RS]])
                nc.sync.dma_start(out=ap, in_=sb_tile[:])

            m1d = dram.tile([1, NQ], F32)
            dump_nmajor(m1c, m1d)
            w1rd = dram.tile([1, NQ], BF16)
            dump_nmajor(w1r_bf, w1rd)
            w2rd = dram.tile([1, NQ], BF16)
            dump_nmajor(w2r_bf, w2rd)
            gin = dram.tile([1, 3 * NQ], F32)
            dump_nmajor(m2c50, gin, 0)
            dump_nmajor(w1cf, gin, NQ)
            dump_nmajor(w2cf, gin, 2 * NQ)
            gout = dram.tile([1, 4 * 3 * NQ], F32)
            if skip_dumps:
                pass
            elif BISECT_NO_COLLECTIVE:
                for qq in range(4):
                    nc.sync.dma_start(
                        out=bass.AP(tensor=gout.tensor,
                                    offset=gout.offset + qq * 3 * NQ,
                                    ap=[[1, 3 * NQ]]),
                        in_=gin[:])
            else:
                nc.gpsimd.collective_compute(
                    "AllGather", ALU.bypass,
                    replica_groups=[[0, 1, 2, 3], [4, 5, 6, 7]],
                    ins=[gin[:]], outs=[gout[:]])

            # broadcasts for the grad pass
            m1b = persist.tile([128, NQ], F32)
            if not skip_dumps:
              nc.sync.dma_start(
                out=m1b[:],
                in_=bass.AP(tensor=m1d.tensor, offset=m1d.offset,
                            ap=[[0, 128], [1, NQ]]))
              w1rb = persist.tile([128, NQ], BF16)
              nc.sync.dma_start(
                  out=w1rb[:],
                  in_=bass.AP(tensor=w1rd.tensor, offset=w1rd.offset,
                              ap=[[0, 128], [1, NQ]]))
              w2rb = persist.tile([128, NQ], BF16)
              nc.sync.dma_start(
                  out=w2rb[:],
                  in_=bass.AP(tensor=w2rd.tensor, offset=w2rd.offset,
                              ap=[[0, 128], [1, NQ]]))
              # gathered col stats: TG[p, k, s_hi, s_lo], m stripe s = 8*s_hi+s_lo
              TG = persist.tile([128, 3, 4, RS], F32)
              for k in range(3):
                  for s_hi in range(4):
                      nc.sync.dma_start(
                          out=TG[:, k, s_hi, :],
                          in_=bass.AP(tensor=gout.tensor,
                                      offset=gout.offset + s_hi * 3 * NQ + k * NQ,
                                      ap=[[1, 128], [128, RS]]))

            # ---- gradient pass: column orientation, 32 stripes [128, 1024] ----
            with ExitStack() as sctx:
              if BISECT_NO_GRAD:
                pass
              else:
                  gp32 = sctx.enter_context(tc.tile_pool(name="g32", bufs=2))
                  gbf = sctx.enter_context(tc.tile_pool(name="gbf", bufs=2))
                  gpsum = sctx.enter_context(
                      tc.tile_pool(name="gps", bufs=2, space="PSUM"))
                  gacc = sctx.enter_context(
                      tc.tile_pool(name="gacc", bufs=1, space="PSUM"))
                  gpts = sctx.enter_context(tc.tile_pool(name="gpts", bufs=1))

                  YTn = gpts.tile([4, M], F32)
                  nc.sync.dma_start(YTn[:], t["YTn"][:])
                  XTg = gpts.tile([4, NQ], F32)
                  nc.sync.dma_start(XTg[:], t["XTg"][:])
                  yyfeT = gpts.tile([128, GS], F32)
                  nc.sync.dma_start(yyfeT[:], t["yyfeT"][:])
                  Y4g = gpts.tile([128, 4 * GS], BF16)
                  nc.sync.dma_start(Y4g[:], t["Y4g"][:])

                  pacc = gacc.tile([4, NQ], F32)

                  for s in range(GS):
                      lhsT = YTn[:, s * 128:(s + 1) * 128]
                      pd2 = gpsum.tile([128, NQ], F32, tag="pd2g")
                      for h in range(2):
                          nc.tensor.matmul(pd2[:, h * 512:(h + 1) * 512], lhsT,
                                           XTg[:, h * 512:(h + 1) * 512],
                                           start=True, stop=True)
                      t1 = gp32.tile([128, NQ], F32, tag="t1g")
                      nc.vector.tensor_scalar(out=t1[:], in0=pd2[:],
                                              scalar1=yyfeT[:, s:s + 1],
                                              scalar2=1e-12,
                                              op0=ALU.add, op1=ALU.max)
                      if BISECT_NO_INPLACE:
                          t1b = gp32.tile([128, NQ], F32, tag="t1gb")
                          nc.scalar.activation(t1b[:], t1[:], AF.Ln)
                          t1 = t1b
                      else:
                          nc.scalar.activation(t1[:], t1[:], AF.Ln)
                      d = gp32.tile([128, NQ], F32, tag="dg")
                      nc.scalar.activation(d[:], t1[:], AF.Exp, scale=0.5)
                      qv = gbf.tile([128, NQ], BF16, tag="qv")
                      nc.scalar.activation(qv[:], t1[:], AF.Exp, scale=-0.5)
                      s_hi, s_lo = s // RS, s % RS
                      ec = gbf.tile([128, NQ], BF16, tag="ec")
                      nc.scalar.activation(ec[:], d[:], AF.Exp,
                                           bias=TG[:, 0, s_hi, s_lo:s_lo + 1],
                                           scale=-INV_TAU)
                      tr = gp32.tile([128, NQ], F32, tag="trg")
                      nc.vector.tensor_tensor(out=tr[:], in0=m1b[:], in1=d[:],
                                              op=ALU.subtract)
                      er = gbf.tile([128, NQ], BF16, tag="er")
                      nc.scalar.activation(er[:], tr[:], AF.Exp, scale=INV_TAU)
                      bc = gbf.tile([128, NQ], BF16, tag="bc")
                      nc.vector.tensor_scalar(out=bc[:], in0=qv[:],
                                              scalar1=TG[:, 1, s_hi, s_lo:s_lo + 1],
                                              scalar2=TG[:, 2, s_hi, s_lo:s_lo + 1],
                                              op0=ALU.mult, op1=ALU.subtract)
                      br = gbf.tile([128, NQ], BF16, tag="br")
                      nc.vector.tensor_tensor(out=br[:], in0=qv[:], in1=w1rb[:],
                                              op=ALU.mult)
                      nc.vector.tensor_tensor(out=br[:], in0=br[:], in1=w2rb[:],
                                              op=ALU.subtract)
                      ha = gbf.tile([128, NQ], BF16, tag="ha")
                      nc.vector.tensor_tensor(out=ha[:], in0=er[:], in1=br[:],
                                              op=ALU.mult)
                      hb = gbf.tile([128, NQ], BF16, tag="hb")
                      nc.vector.tensor_tensor(out=hb[:], in0=ec[:], in1=bc[:],
                                              op=ALU.mult)
                      for h in range(2):
                          nc.tensor.matmul(pacc[:, h * 512:(h + 1) * 512],
                                           Y4g[:, 4 * s:4 * s + 4],
                                           ha[:, h * 512:(h + 1) * 512],
                                           start=(s == 0), stop=False,
                                           skip_group_check=True)
                          nc.tensor.matmul(pacc[:, h * 512:(h + 1) * 512],
                                           Y4g[:, 4 * s:4 * s + 4],
                                           hb[:, h * 512:(h + 1) * 512],
                                           start=False, stop=(s == GS - 1),
                                           skip_group_check=True)

                  # finalize gradient: g = x*S - HY ; |norm(g)-1| partials
                  gsb = gpts.tile([4, NQ], F32)
                  nc.scalar.copy(gsb[:], pacc[:])
                  gd = dram.tile([1, 4 * NQ], F32)
                  nc.sync.dma_start(
                      out=bass.AP(tensor=gd.tensor, offset=gd.offset,
                                  ap=[[NQ, 4], [1, NQ]]),
                      in_=gsb[:])
                  gt = gpts.tile([128, 4, RS], F32)
                  for k in range(4):
                      nc.sync.dma_start(
                          out=gt[:, k, :],
                          in_=bass.AP(tensor=gd.tensor, offset=gd.offset + k * NQ,
                                      ap=[[1, 128], [128, RS]]))
                  xq = []
                  for k in range(3):
                      xk = gpts.tile([128, RS], F32)
                      nc.sync.dma_start(xk[:], t["xq%d" % k][:])
                      xq.append(xk)
                  S_ap = gt[:, 3:4, :].rearrange("p a b -> p (a b)")
                  n2 = gpts.tile([128, RS], F32)
                  sq = gpts.tile([128, RS], F32)
                  gk = gpts.tile([128, RS], F32)
                  for k in range(3):
                      hy_ap = gt[:, k:k + 1, :].rearrange("p a b -> p (a b)")
                      nc.vector.tensor_tensor(out=gk[:], in0=xq[k][:], in1=S_ap,
                                              op=ALU.mult)
                      nc.vector.tensor_tensor(out=gk[:], in0=gk[:], in1=hy_ap,
                                              op=ALU.subtract)
                      nc.scalar.activation(sq[:], gk[:], AF.Square)
                      if k == 0:
                          nc.vector.tensor_copy(out=n2[:], in_=sq[:])
                      else:
                          nc.vector.tensor_tensor(out=n2[:], in0=n2[:], in1=sq[:],
                                                  op=ALU.add)
                  # ||g|| = exp(0.5 ln n2); dev = ||g|| - 1; |dev| -> out cols 32:40
                  nc.scalar.activation(sq[:], n2[:], AF.Ln)
                  nc.scalar.activation(n2[:], sq[:], AF.Exp, scale=0.5)
                  nc.vector.tensor_scalar(out=sq[:], in0=n2[:], scalar1=1.0,
                                          scalar2=None, op0=ALU.subtract)
                  nc.scalar.activation(outsb[:, 32:40], sq[:], AF.Abs)

            nc.sync.dma_start(out[:], outsb[:])

    nc.compile()
    return nc


def _get_nc():
    if "nc" not in _CACHE:
        _CACHE["nc"] = _build()
    return _CACHE["nc"]


def _pack_core_inputs(inputs):
    """Slice/transform full inputs into 8 per-core input maps (+host context)."""
    in_maps = []
    host = []
    flat = {k: np.ascontiguousarray(inputs[k]).reshape(B * P)
            for k in ["sdf_pred", "sdf_target", "uncertainty",
                      "occupancy_pred", "occupancy_target"]}
    nrm_a = np.ascontiguousarray(inputs["normals_pred"]).reshape(B * P, 3)
    nrm_b = np.ascontiguousarray(inputs["normals_target"]).reshape(B * P, 3)
    pcp = np.asarray(inputs["point_cloud_pred"], dtype=np.float32)
    pct = np.asarray(inputs["point_cloud_target"], dtype=np.float32)

    for c in range(8):
        b, q = c // 4, c % 4
        lo = c * (B * P // 8)
        hi = lo + B * P // 8
        m = {}
        m["sdfa"] = flat["sdf_pred"][lo:hi].reshape(EWP, EWF)
        m["sdfb"] = flat["sdf_target"][lo:hi].reshape(EWP, EWF)
        m["uncw"] = flat["uncertainty"][lo:hi].reshape(EWP, EWF)
        m["occp"] = flat["occupancy_pred"][lo:hi].reshape(EWP, EWF)
        m["occt"] = flat["occupancy_target"][lo:hi].reshape(EWP, EWF)
        for k, nm in enumerate(["nax", "nay", "naz"]):
            m[nm] = np.ascontiguousarray(nrm_a[lo:hi, k]).reshape(EWP, EWF)
        for k, nm in enumerate(["nbx", "nby", "nbz"]):
            m[nm] = np.ascontiguousarray(nrm_b[lo:hi, k]).reshape(EWP, EWF)

        X = pcp[b]          # [N,3]
        Y = pct[b]          # [M,3]
        xx = (X * X).sum(1).astype(np.float32)
        yy = (Y * Y).sum(1).astype(np.float32)
        rows = slice(q * NQ, (q + 1) * NQ)
        Xq = X[rows]

        m["XTq"] = np.ascontiguousarray(
            np.vstack([-2.0 * Xq.T, np.ones((1, NQ), np.float32)]))
        m["YTm"] = np.ascontiguousarray(np.vstack([Y.T, yy[None, :]]))
        Yq = Y[rows]
        m["YTnq"] = np.ascontiguousarray(
            np.vstack([-2.0 * Yq.T, np.ones((1, NQ), np.float32)]))
        m["YTn"] = np.ascontiguousarray(
            np.vstack([-2.0 * Y.T, np.ones((1, M), np.float32)]))
        m["XTm"] = np.ascontiguousarray(np.vstack([X.T, xx[None, :]]))
        m["XTg"] = np.ascontiguousarray(m["XTm"][:, rows])
        m["xxqeT"] = np.ascontiguousarray(xx[rows].reshape(RS, 128).T)
        m["yyqeT"] = np.ascontiguousarray(yy[rows].reshape(RS, 128).T)
        m["yyfeT"] = np.ascontiguousarray(yy.reshape(GS, 128).T)
        y4 = np.concatenate([Y.reshape(GS, 128, 3),
                             np.ones((GS, 128, 1), np.float32)], axis=2)
        # Y4g[p, 4s+k] = y4[s, p, k]
        m["Y4g"] = _to_bf16(np.ascontiguousarray(
            y4.transpose(1, 0, 2).reshape(128, 4 * GS)))
        for k in range(3):
            m["xq%d" % k] = np.ascontiguousarray(
                Xq[:, k].reshape(RS, 128).T)
        in_maps.append(m)
        host.append(dict(xxq=xx[rows].reshape(RS, 128),
                         yyq=yy[rows].reshape(RS, 128)))
    return in_maps, host


def _to_bf16(a):
    import ml_dtypes
    return a.astype(ml_dtypes.bfloat16)


def _combine(results, host):
    """Host-side combine of per-core [128,64] partial tiles (float64)."""
    s_l1 = s_b1 = s_b2 = s_cos = s_unc = 0.0
    s_rowmin = s_colmin = s_A = s_C = s_gp = 0.0
    for c in range(8):
        o = results[c]["out"].astype(np.float64)
        # chamfer mins arrive clamped with xx/yy included; sqrt in f64
        s_rowmin += np.sqrt(np.maximum(o[:, 0:RS], 1e-12)).sum()
        s_colmin += np.sqrt(np.maximum(o[:, 16:16 + RS], 1e-12)).sum()
        s_A += o[:, 8:16].sum()
        s_C += o[:, 24:32].sum()
        s_gp += o[:, 32:40].sum()
        s_l1 += o[0:EWP, 40].sum()
        s_b1 += o[0:EWP, 41].sum()
        s_b2 += o[0:EWP, 42].sum()
        s_cos += o[0:EWP, 43].sum()
        s_unc += o[0:EWP, 44].sum()

    sdf_loss = s_l1 / (B * P)
    bce = -(s_b1 + s_b2) / (B * P)
    p_t = np.exp(-bce)
    occ_loss = FOCAL_ALPHA * (1.0 - p_t) ** FOCAL_GAMMA * bce
    normal_loss = 1.0 - s_cos / (B * P)
    unc_reg = s_unc / (B * P)
    chamfer = s_rowmin / (B * N) + s_colmin / (B * M)
    emd = (s_A / N + s_C / M) / B
    gp = s_gp / (B * N)
    total = (W_SDF * sdf_loss + W_OCC * occ_loss + W_NORMAL * normal_loss
             + W_CHAMFER * chamfer + W_EMD * emd + W_EMD_GP * gp
             + W_UNC * unc_reg)
    return np.float32(total)


def run(inputs, trace=False):
    nc = _get_nc()
    in_maps, host = _pack_core_inputs(inputs)
    res = run_bass_kernel_spmd(nc, in_maps, list(range(8)), trace=trace)
    total = _combine(res.results, host)
    return total, res


def kernel(**inputs) -> np.ndarray:
    total, _ = run(inputs)
    return np.asarray(total, dtype=np.float32)



# revision 3
# speedup vs baseline: 1.1589x; 1.1589x over previous
"""Trainium2 Bass kernel for nn_AdvancedLossFunction (8-core SPMD).

Sharding: 8 cores = 2 batches x 4 quarters. Core c handles batch b=c//4,
row/col quarter q=c%4 of the 4096x4096 point-cloud distance matrix, plus
1/8 of the elementwise (P=100000) losses.

Math (validated against the jax reference in a numpy prototype):
  chamfer = mean rowmin d + mean colmin d
  emd     = mean_b [mean_n A_n + mean_m C_m],  A_n = sum_m r*d (row softmax)
  gp      = mean |  ||g|| - 1 |,  g = d(emd)/d(pred) computed analytically:
      g_n = x_n * S_n - (H @ Y)_n,  H = G/d,
      G = 1/(B*N) r*(1-(d-A)/tau) + 1/(B*M) c*(1-(d-C)/tau)

Device strategy:
  - d^2 via PE matmuls (k=4 trick: lhsT=[-2x|1], rhs=[y|yy], + xx as ACT bias)
  - sqrt & 1/x via exp(+-0.5*ln(.)) so ALL transcendentals live in one ACT
    table set (natural_log_exp_and_others) -> zero table switches
  - row/col passes fp32; col stats AllGathered across the 4 cores of a batch
  - gradient pass in column orientation (m on partitions): coefficients are
    per-partition scalars, reductions via PE matmul with lhsT=[y0,y1,y2,1],
    elementwise H products in bf16 (gp term is error-tolerant: ||g||<<1)
  - each core outputs a [128,64] tile of partial sums; host combines in f64
"""
import numpy as np
from contextlib import ExitStack

import concourse.bass as bass
import concourse.bacc as bacc
import concourse.tile as tile
from concourse import mybir
from concourse.bass_utils import run_bass_kernel_spmd

F32 = mybir.dt.float32
F32R = mybir.dt.float32r
BF16 = mybir.dt.bfloat16
AF = mybir.ActivationFunctionType
ALU = mybir.AluOpType


class _PinnedBacc(bacc.Bacc):
    """Bacc that pins all activations to the natural_log_exp_and_others
    table set (ln+exp+abs+square+copy in one set) so the compiler never
    thrashes between the per-function preferred sets."""

    _PIN_SET = "natural_log_exp_and_others"

    def insert_act_table_loads(self):
        has_activation = any(
            isinstance(i, mybir.InstActivation)
            for b in self.main_func.blocks
            for i in b.instructions
        )
        if not has_activation:
            return
        from concourse.hw_specs import get_activation_tables
        import bass_rust as _br

        tables = []
        for name, funcs in get_activation_tables(self.m.arch).items():
            tables.append((name, funcs if name == self._PIN_SET else set()))
        _br.insert_act_table_loads(self, tables)

TAU = 0.02
INV_TAU = 50.0
B, P = 2, 100000
N = M = 4096
NQ = 1024          # quarter size
RS = NQ // 128     # 8 row stripes per quarter
GS = M // 128      # 32 grad stripes (all m)
EPS_D2 = 4e-5      # ln-domain clamp; folded into the xx/yy bias on host
EWP, EWF = 125, 200  # per-core elementwise slice 25000 = 125*200

FOCAL_GAMMA, FOCAL_ALPHA = 2.0, 0.75
W_SDF, W_OCC, W_NORMAL, W_CHAMFER, W_EMD, W_EMD_GP, W_UNC = \
    1.0, 1.0, 0.1, 1.0, 0.25, 0.05, 0.1

_CACHE = {}

import os
BISECT_NO_COLLECTIVE = bool(int(os.environ.get("K_NO_COLLECTIVE", "0")))
BISECT_NO_GRAD = bool(int(os.environ.get("K_NO_GRAD", "0")))
BISECT_NO_INPLACE = bool(int(os.environ.get("K_NO_INPLACE", "0")))
BISECT_NO_DUMPS = bool(int(os.environ.get("K_NO_DUMPS", "0")))
BISECT_NO_STATS = bool(int(os.environ.get("K_NO_STATS", "0")))
BISECT_NO_EW = bool(int(os.environ.get("K_NO_EW", "0")))


def _declare_inputs(nc):
    t = {}
    for name in ["sdfa", "sdfb", "uncw", "occp", "occt",
                 "nax", "nay", "naz", "nbx", "nby", "nbz"]:
        t[name] = nc.dram_tensor(name, [EWP, EWF], F32, kind="ExternalInput")
    t["XTq"] = nc.dram_tensor("XTq", [4, NQ], F32R, kind="ExternalInput")
    t["YTm"] = nc.dram_tensor("YTm", [4, M], F32R, kind="ExternalInput")
    t["YTnq"] = nc.dram_tensor("YTnq", [4, NQ], F32R, kind="ExternalInput")
    t["YTn"] = nc.dram_tensor("YTn", [4, M], F32R, kind="ExternalInput")
    t["XTm"] = nc.dram_tensor("XTm", [4, N], F32R, kind="ExternalInput")
    t["XTg"] = nc.dram_tensor("XTg", [4, NQ], F32R, kind="ExternalInput")
    t["xxqeT"] = nc.dram_tensor("xxqeT", [128, RS], F32, kind="ExternalInput")
    t["yyqeT"] = nc.dram_tensor("yyqeT", [128, RS], F32, kind="ExternalInput")
    t["yyfeT"] = nc.dram_tensor("yyfeT", [128, GS], F32, kind="ExternalInput")
    t["Y4g"] = nc.dram_tensor("Y4g", [128, 4 * GS], BF16, kind="ExternalInput")
    t["xq0"] = nc.dram_tensor("xq0", [128, RS], F32, kind="ExternalInput")
    t["xq1"] = nc.dram_tensor("xq1", [128, RS], F32, kind="ExternalInput")
    t["xq2"] = nc.dram_tensor("xq2", [128, RS], F32, kind="ExternalInput")
    return t


def _ew_stage(nc, tc, ctx, t, outsb, pool):
    """Elementwise losses on the core's 25000-point slice -> outsb cols 40-44."""
    def load(name):
        s = pool.tile([EWP, EWF], F32, tag="ewin_" + name)
        nc.sync.dma_start(s[:], t[name][:])
        return s

    sdfa, sdfb = load("sdfa"), load("sdfb")
    # L1: |a-b| summed
    diff = pool.tile([EWP, EWF], F32, tag="ewt0")
    nc.vector.tensor_tensor(out=diff[:], in0=sdfa[:], in1=sdfb[:], op=ALU.subtract)
    junk = pool.tile([EWP, EWF], F32, tag="ewt1")
    nc.scalar.activation(junk[:], diff[:], AF.Abs, accum_out=outsb[0:EWP, 40:41])

    occp, occt = load("occp"), load("occt")
    lnp = pool.tile([EWP, EWF], F32, tag="ewt2")
    nc.scalar.activation(lnp[:], occp[:], AF.Ln)
    j2 = pool.tile([EWP, EWF], F32, tag="ewt3")
    nc.vector.tensor_tensor(out=j2[:], in0=occt[:], in1=lnp[:], op=ALU.mult)
    jr = pool.tile([EWP, EWF], F32, tag="ewtr")
    nc.vector.tensor_scalar(out=jr[:], in0=j2[:], scalar1=1.0, scalar2=None,
                            op0=ALU.mult, op1=ALU.add, accum_out=outsb[0:EWP, 41:42])
    onemp = pool.tile([EWP, EWF], F32, tag="ewt4")
    nc.vector.tensor_scalar(out=onemp[:], in0=occp[:], scalar1=-1.0, scalar2=1.0,
                            op0=ALU.mult, op1=ALU.add)
    ln1mp = pool.tile([EWP, EWF], F32, tag="ewt5")
    nc.scalar.activation(ln1mp[:], onemp[:], AF.Ln)
    onemt = pool.tile([EWP, EWF], F32, tag="ewt6")
    nc.vector.tensor_scalar(out=onemt[:], in0=occt[:], scalar1=-1.0, scalar2=1.0,
                            op0=ALU.mult, op1=ALU.add)
    nc.vector.tensor_tensor(out=lnp[:], in0=onemt[:], in1=ln1mp[:], op=ALU.mult)
    nc.vector.tensor_scalar(out=jr[:], in0=lnp[:], scalar1=1.0, scalar2=None,
                            op0=ALU.mult, op1=ALU.add, accum_out=outsb[0:EWP, 42:43])

    # cosine similarity of normals
    nax, nay, naz = load("nax"), load("nay"), load("naz")
    nbx, nby, nbz = load("nbx"), load("nby"), load("nbz")
    dot = pool.tile([EWP, EWF], F32, tag="ewt0")
    nc.vector.tensor_tensor(out=dot[:], in0=nax[:], in1=nbx[:], op=ALU.mult)
    tmp = pool.tile([EWP, EWF], F32, tag="ewt1")
    nc.vector.tensor_tensor(out=tmp[:], in0=nay[:], in1=nby[:], op=ALU.mult)
    nc.vector.tensor_tensor(out=dot[:], in0=dot[:], in1=tmp[:], op=ALU.add)
    nc.vector.tensor_tensor(out=tmp[:], in0=naz[:], in1=nbz[:], op=ALU.mult)
    nc.vector.tensor_tensor(out=dot[:], in0=dot[:], in1=tmp[:], op=ALU.add)

    def norm2(cx, cy, cz, tag):
        n2 = pool.tile([EWP, EWF], F32, tag=tag)
        s1 = pool.tile([EWP, EWF], F32, tag=tag + "s")
        nc.scalar.activation(n2[:], cx[:], AF.Square)
        nc.scalar.activation(s1[:], cy[:], AF.Square)
        nc.vector.tensor_tensor(out=n2[:], in0=n2[:], in1=s1[:], op=ALU.add)
        nc.scalar.activation(s1[:], cz[:], AF.Square)
        nc.vector.tensor_tensor(out=n2[:], in0=n2[:], in1=s1[:], op=ALU.add)
        # 1/norm = exp(-0.5*ln(n2))
        nc.scalar.activation(s1[:], n2[:], AF.Ln)
        nc.scalar.activation(n2[:], s1[:], AF.Exp, scale=-0.5)
        return n2

    ra = norm2(nax, nay, naz, "ewt2")
    rb = norm2(nbx, nby, nbz, "ewt4")
    nc.vector.tensor_tensor(out=dot[:], in0=dot[:], in1=ra[:], op=ALU.mult)
    nc.vector.tensor_tensor(out=dot[:], in0=dot[:], in1=rb[:], op=ALU.mult)
    nc.vector.tensor_scalar(out=tmp[:], in0=dot[:], scalar1=1.0, scalar2=None,
                            op0=ALU.mult, op1=ALU.add, accum_out=outsb[0:EWP, 43:44])

    # uncertainty regularizer u*(1-u)
    uncw = load("uncw")
    onemu = pool.tile([EWP, EWF], F32, tag="ewt0")
    nc.vector.tensor_scalar(out=onemu[:], in0=uncw[:], scalar1=-1.0, scalar2=1.0,
                            op0=ALU.mult, op1=ALU.add)
    nc.vector.tensor_tensor(out=onemu[:], in0=uncw[:], in1=onemu[:], op=ALU.mult)
    nc.vector.tensor_scalar(out=tmp[:], in0=onemu[:], scalar1=1.0, scalar2=None,
                            op0=ALU.mult, op1=ALU.add, accum_out=outsb[0:EWP, 44:45])


def _stats_pass(nc, tc, ctx, lhsT_sb, rhs_sb, bias_sb, pools, outsb, outcol,
                minraw_t, mc_t, ssum, tsum):
    """One stats pass (row or col orientation): 8 stripes of [128, 4096].

    Computes per-stripe: raw min of (psum) [host adds xx for chamfer],
    clamped min mc, softmax sum s (-> ssum) and weighted sum t (-> tsum).
    """
    sbuf, psum = pools
    CH = 2048
    for i in range(RS):
        lhsT = lhsT_sb[:, i * 128:(i + 1) * 128]
        bias = bias_sb[:, i:i + 1]
        t1 = sbuf.tile([128, M], F32, tag="t1")
        for h in range(2):
            pd2 = psum.tile([128, CH], F32, tag="pd2")
            for j in range(4):
                nc.tensor.matmul(
                    pd2[:, j * 512:(j + 1) * 512], lhsT,
                    rhs_sb[:, h * CH + j * 512: h * CH + (j + 1) * 512],
                    start=True, stop=True)
            # exact reference clamp: d2 = max(psum + xx, 1e-12)  [PSUM -> SBUF]
            nc.vector.tensor_scalar(out=t1[:, h * CH:(h + 1) * CH], in0=pd2[:],
                                    scalar1=bias, scalar2=1e-12,
                                    op0=ALU.add, op1=ALU.max)
        # clamped min d2 (host takes sqrt in f64 for chamfer)
        nc.vector.tensor_reduce(out=minraw_t[:, i:i + 1], in_=t1[:],
                                axis=mybir.AxisListType.X, op=ALU.min)
        # mc = exp(0.5*ln(min d2)); bias50 = 50*mc
        lmin = sbuf.tile([128, 1], F32, tag="lmin")
        nc.scalar.activation(lmin[:], minraw_t[:, i:i + 1], AF.Ln)
        nc.scalar.activation(mc_t[:, i:i + 1], lmin[:], AF.Exp, scale=0.5)
        # t1 = ln(d2) (in place unless bisecting)
        if BISECT_NO_INPLACE:
            t1b = sbuf.tile([128, M], F32, tag="t1b")
            nc.scalar.activation(t1b[:], t1[:], AF.Ln)
            t1 = t1b
        else:
            nc.scalar.activation(t1[:], t1[:], AF.Ln)
        b50 = sbuf.tile([128, 1], F32, tag="b50")
        nc.scalar.mul(b50[:], mc_t[:, i:i + 1], INV_TAU)
        # d = exp(0.5 * t1)
        d = sbuf.tile([128, M], F32, tag="dstripe")
        nc.scalar.activation(d[:], t1[:], AF.Exp, scale=0.5)
        # e = exp(-50 d + 50 mc), s = sum e
        er = sbuf.tile([128, M], F32, tag="erstripe")
        nc.scalar.activation(er[:], d[:], AF.Exp, bias=b50[:], scale=-INV_TAU,
                             accum_out=ssum[:, i:i + 1])
        # t = sum e*d  (TT product + TS accumulate; TTR is broken on HW here)
        junk = sbuf.tile([128, M], F32, tag="junkstripe")
        nc.vector.tensor_tensor(out=junk[:], in0=er[:], in1=d[:], op=ALU.mult)
        nc.vector.tensor_scalar(out=er[:], in0=junk[:], scalar1=1.0,
                                scalar2=None, op0=ALU.mult, op1=ALU.add,
                                accum_out=tsum[:, i:i + 1])


def _build(trn_type="TRN2"):
    nc = _PinnedBacc(trn_type, target_bir_lowering=False)
    t = _declare_inputs(nc)
    out = nc.dram_tensor("out", [128, 64], F32, kind="ExternalOutput")

    with tile.TileContext(nc) as tc:
        with ExitStack() as ctx:
            persist = ctx.enter_context(tc.tile_pool(name="persist", bufs=1))
            dram = ctx.enter_context(tc.tile_pool(name="dram", bufs=1, space="DRAM"))

            outsb = persist.tile([128, 64], F32)
            nc.vector.memset(outsb[:], 0.0)

            # persistent stats tiles [128, RS] (stripe-major: n = i*128 + p)
            m1raw = persist.tile([128, RS], F32)   # raw row min(psum)
            m1c = persist.tile([128, RS], F32)     # clamped row min
            ssum = persist.tile([128, RS], F32)
            tsum = persist.tile([128, RS], F32)
            m2raw = persist.tile([128, RS], F32)
            m2c = persist.tile([128, RS], F32)
            usum = persist.tile([128, RS], F32)
            vsum = persist.tile([128, RS], F32)

            # ---- elementwise + stats passes ----
            if not BISECT_NO_EW:
                with ExitStack() as sctx:
                    ewpool = sctx.enter_context(tc.tile_pool(name="ew", bufs=1))
                    _ew_stage(nc, tc, sctx, t, outsb, ewpool)

            with ExitStack() as sctx:
                spool = sctx.enter_context(tc.tile_pool(name="sp", bufs=2))
                ppool = sctx.enter_context(
                    tc.tile_pool(name="pp", bufs=2, space="PSUM"))
                pts = sctx.enter_context(tc.tile_pool(name="pts", bufs=1))

                XTq = pts.tile([4, NQ], F32R)
                nc.sync.dma_start(XTq[:], t["XTq"][:])
                YTm = pts.tile([4, M], F32R)
                nc.sync.dma_start(YTm[:], t["YTm"][:])
                YTnq = pts.tile([4, NQ], F32R)
                nc.sync.dma_start(YTnq[:], t["YTnq"][:])
                XTm = pts.tile([4, N], F32R)
                nc.sync.dma_start(XTm[:], t["XTm"][:])
                xxqeT = pts.tile([128, RS], F32)
                nc.sync.dma_start(xxqeT[:], t["xxqeT"][:])
                yyqeT = pts.tile([128, RS], F32)
                nc.sync.dma_start(yyqeT[:], t["yyqeT"][:])

                # row pass: quarter rows x all m
                if not BISECT_NO_STATS:
                    _stats_pass(nc, tc, sctx, XTq, YTm, xxqeT, (spool, ppool),
                                outsb, 0, m1raw, m1c, ssum, tsum)
                    # col pass: quarter cols x all n
                    _stats_pass(nc, tc, sctx, YTnq, XTm, yyqeT, (spool, ppool),
                                outsb, 16, m2raw, m2c, usum, vsum)
                else:
                    for tt in (m1raw, m1c, ssum, tsum, m2raw, m2c, usum, vsum):
                        nc.vector.memset(tt[:], 1.0)

            # raw mins to output (host finishes chamfer in f64)
            nc.vector.tensor_copy(out=outsb[:, 0:RS], in_=m1raw[:])
            nc.vector.tensor_copy(out=outsb[:, 16:16 + RS], in_=m2raw[:])

            # ---- stats finalize: A, C, softmax coefficients ----
            fin = ctx.enter_context(tc.tile_pool(name="fin", bufs=1))
            sinv = fin.tile([128, RS], F32)
            nc.vector.reciprocal(out=sinv[:], in_=ssum[:])
            uinv = fin.tile([128, RS], F32)
            nc.vector.reciprocal(out=uinv[:], in_=usum[:])
            # A = t/s -> outsb[:, 8:16];  C = v/u -> outsb[:, 24:32]
            nc.vector.tensor_tensor(out=outsb[:, 8:16], in0=tsum[:], in1=sinv[:],
                                    op=ALU.mult)
            nc.vector.tensor_tensor(out=outsb[:, 24:32], in0=vsum[:], in1=uinv[:],
                                    op=ALU.mult)
            # w1r = (1+50A)/(B*N*s); w2r = 50/(B*N*s)   (bf16 for grad pass)
            scr = fin.tile([128, RS], F32)
            nc.vector.tensor_scalar(out=scr[:], in0=outsb[:, 8:16], scalar1=INV_TAU,
                                    scalar2=1.0, op0=ALU.mult, op1=ALU.add)
            w1rf = fin.tile([128, RS], F32)
            nc.vector.tensor_tensor(out=w1rf[:], in0=scr[:], in1=sinv[:], op=ALU.mult)
            w1r_bf = fin.tile([128, RS], BF16)
            nc.vector.tensor_scalar(out=w1r_bf[:], in0=w1rf[:],
                                    scalar1=1.0 / (B * N), scalar2=None, op0=ALU.mult)
            w2r_bf = fin.tile([128, RS], BF16)
            nc.vector.tensor_scalar(out=w2r_bf[:], in0=sinv[:],
                                    scalar1=INV_TAU / (B * N), scalar2=None,
                                    op0=ALU.mult)
            # col coefficients (f32, gathered below)
            nc.vector.tensor_scalar(out=scr[:], in0=outsb[:, 24:32], scalar1=INV_TAU,
                                    scalar2=1.0, op0=ALU.mult, op1=ALU.add)
            w1cf = fin.tile([128, RS], F32)
            nc.vector.tensor_tensor(out=w1cf[:], in0=scr[:], in1=uinv[:], op=ALU.mult)
            nc.vector.tensor_scalar(out=w1cf[:], in0=w1cf[:], scalar1=1.0 / (B * M),
                                    scalar2=None, op0=ALU.mult)
            w2cf = fin.tile([128, RS], F32)
            nc.vector.tensor_scalar(out=w2cf[:], in0=uinv[:],
                                    scalar1=INV_TAU / (B * M), scalar2=None,
                                    op0=ALU.mult)
            m2c50 = fin.tile([128, RS], F32)
            nc.scalar.mul(m2c50[:], m2c[:], INV_TAU)

            # ---- DRAM roundtrips: broadcasts + AllGather of col stats ----
            skip_dumps = BISECT_NO_DUMPS or BISECT_NO_STATS
            def dump_nmajor(sb_tile, dram_tile, off=0):
                if skip_dumps:
                    return
                # SBUF [128, RS] -> DRAM flat [NQ] with index i*128+p
                ap = bass.AP(tensor=dram_tile.tensor, offset=dram_tile.offset + off,
                             ap=[[1, 128], [128, RS]])
                nc.sync.dma_start(out=ap, in_=sb_tile[:])

            m1d = dram.tile([1, NQ], F32)
            dump_nmajor(m1c, m1d)
            w1rd = dram.tile([1, NQ], BF16)
            dump_nmajor(w1r_bf, w1rd)
            w2rd = dram.tile([1, NQ], BF16)
            dump_nmajor(w2r_bf, w2rd)
            gin = dram.tile([1, 3 * NQ], F32)
            dump_nmajor(m2c50, gin, 0)
            dump_nmajor(w1cf, gin, NQ)
            dump_nmajor(w2cf, gin, 2 * NQ)
            gout = dram.tile([1, 4 * 3 * NQ], F32)
            if skip_dumps:
                pass
            elif BISECT_NO_COLLECTIVE:
                for qq in range(4):
                    nc.sync.dma_start(
                        out=bass.AP(tensor=gout.tensor,
                                    offset=gout.offset + qq * 3 * NQ,
                                    ap=[[1, 3 * NQ]]),
                        in_=gin[:])
            else:
                nc.gpsimd.collective_compute(
                    "AllGather", ALU.bypass,
                    replica_groups=[[0, 1, 2, 3], [4, 5, 6, 7]],
                    ins=[gin[:]], outs=[gout[:]])

            # broadcasts for the grad pass
            m1b = persist.tile([128, NQ], F32)
            if not skip_dumps:
              nc.sync.dma_start(
                out=m1b[:],
                in_=bass.AP(tensor=m1d.tensor, offset=m1d.offset,
                            ap=[[0, 128], [1, NQ]]))
              w1rb = persist.tile([128, NQ], BF16)
              nc.sync.dma_start(
                  out=w1rb[:],
                  in_=bass.AP(tensor=w1rd.tensor, offset=w1rd.offset,
                              ap=[[0, 128], [1, NQ]]))
              w2rb = persist.tile([128, NQ], BF16)
              nc.sync.dma_start(
                  out=w2rb[:],
                  in_=bass.AP(tensor=w2rd.tensor, offset=w2rd.offset,
                              ap=[[0, 128], [1, NQ]]))
              # gathered col stats: TG[p, k, s_hi, s_lo], m stripe s = 8*s_hi+s_lo
              TG = persist.tile([128, 3, 4, RS], F32)
              for k in range(3):
                  for s_hi in range(4):
                      nc.sync.dma_start(
                          out=TG[:, k, s_hi, :],
                          in_=bass.AP(tensor=gout.tensor,
                                      offset=gout.offset + s_hi * 3 * NQ + k * NQ,
                                      ap=[[1, 128], [128, RS]]))

            # ---- gradient pass: column orientation, 32 stripes [128, 1024] ----
            with ExitStack() as sctx:
              if BISECT_NO_GRAD:
                pass
              else:
                  gp32 = sctx.enter_context(tc.tile_pool(name="g32", bufs=2))
                  gbf = sctx.enter_context(tc.tile_pool(name="gbf", bufs=2))
                  gpsum = sctx.enter_context(
                      tc.tile_pool(name="gps", bufs=2, space="PSUM"))
                  gacc = sctx.enter_context(
                      tc.tile_pool(name="gacc", bufs=1, space="PSUM"))
                  gpts = sctx.enter_context(tc.tile_pool(name="gpts", bufs=1))

                  YTn = gpts.tile([4, M], F32R)
                  nc.sync.dma_start(YTn[:], t["YTn"][:])
                  XTg = gpts.tile([4, NQ], F32R)
                  nc.sync.dma_start(XTg[:], t["XTg"][:])
                  yyfeT = gpts.tile([128, GS], F32)
                  nc.sync.dma_start(yyfeT[:], t["yyfeT"][:])
                  Y4g = gpts.tile([128, 4 * GS], BF16)
                  nc.sync.dma_start(Y4g[:], t["Y4g"][:])

                  pacc = gacc.tile([4, NQ], F32)

                  for s in range(GS):
                      lhsT = YTn[:, s * 128:(s + 1) * 128]
                      pd2 = gpsum.tile([128, NQ], F32, tag="pd2g")
                      for h in range(2):
                          nc.tensor.matmul(pd2[:, h * 512:(h + 1) * 512], lhsT,
                                           XTg[:, h * 512:(h + 1) * 512],
                                           start=True, stop=True)
                      t1 = gp32.tile([128, NQ], F32, tag="t1g")
                      nc.vector.tensor_scalar(out=t1[:], in0=pd2[:],
                                              scalar1=yyfeT[:, s:s + 1],
                                              scalar2=1e-12,
                                              op0=ALU.add, op1=ALU.max)
                      if BISECT_NO_INPLACE:
                          t1b = gp32.tile([128, NQ], F32, tag="t1gb")
                          nc.scalar.activation(t1b[:], t1[:], AF.Ln)
                          t1 = t1b
                      else:
                          nc.scalar.activation(t1[:], t1[:], AF.Ln)
                      d = gp32.tile([128, NQ], F32, tag="dg")
                      nc.scalar.activation(d[:], t1[:], AF.Exp, scale=0.5)
                      qv = gbf.tile([128, NQ], BF16, tag="qv")
                      nc.scalar.activation(qv[:], t1[:], AF.Exp, scale=-0.5)
                      s_hi, s_lo = s // RS, s % RS
                      ec = gbf.tile([128, NQ], BF16, tag="ec")
                      nc.scalar.activation(ec[:], d[:], AF.Exp,
                                           bias=TG[:, 0, s_hi, s_lo:s_lo + 1],
                                           scale=-INV_TAU)
                      tr = gp32.tile([128, NQ], F32, tag="trg")
                      nc.vector.tensor_tensor(out=tr[:], in0=m1b[:], in1=d[:],
                                              op=ALU.subtract)
                      er = gbf.tile([128, NQ], BF16, tag="er")
                      nc.scalar.activation(er[:], tr[:], AF.Exp, scale=INV_TAU)
                      bc = gbf.tile([128, NQ], BF16, tag="bc")
                      nc.vector.tensor_scalar(out=bc[:], in0=qv[:],
                                              scalar1=TG[:, 1, s_hi, s_lo:s_lo + 1],
                                              scalar2=TG[:, 2, s_hi, s_lo:s_lo + 1],
                                              op0=ALU.mult, op1=ALU.subtract)
                      br = gbf.tile([128, NQ], BF16, tag="br")
                      nc.vector.tensor_tensor(out=br[:], in0=qv[:], in1=w1rb[:],
                                              op=ALU.mult)
                      nc.vector.tensor_tensor(out=br[:], in0=br[:], in1=w2rb[:],
                                              op=ALU.subtract)
                      ha = gbf.tile([128, NQ], BF16, tag="ha")
                      nc.vector.tensor_tensor(out=ha[:], in0=er[:], in1=br[:],
                                              op=ALU.mult)
                      hb = gbf.tile([128, NQ], BF16, tag="hb")
                      nc.vector.tensor_tensor(out=hb[:], in0=ec[:], in1=bc[:],
                                              op=ALU.mult)
                      for h in range(2):
                          nc.tensor.matmul(pacc[:, h * 512:(h + 1) * 512],
                                           Y4g[:, 4 * s:4 * s + 4],
                                           ha[:, h * 512:(h + 1) * 512],
                                           start=(s == 0), stop=False,
                                           skip_group_check=True)
                          nc.tensor.matmul(pacc[:, h * 512:(h + 1) * 512],
                                           Y4g[:, 4 * s:4 * s + 4],
                                           hb[:, h * 512:(h + 1) * 512],
                                           start=False, stop=(s == GS - 1),
                                           skip_group_check=True)

                  # finalize gradient: g = x*S - HY ; |norm(g)-1| partials
                  gsb = gpts.tile([4, NQ], F32)
                  nc.scalar.copy(gsb[:], pacc[:])
                  gd = dram.tile([1, 4 * NQ], F32)
                  nc.sync.dma_start(
                      out=bass.AP(tensor=gd.tensor, offset=gd.offset,
                                  ap=[[NQ, 4], [1, NQ]]),
                      in_=gsb[:])
                  gt = gpts.tile([128, 4, RS], F32)
                  for k in range(4):
                      nc.sync.dma_start(
                          out=gt[:, k, :],
                          in_=bass.AP(tensor=gd.tensor, offset=gd.offset + k * NQ,
                                      ap=[[1, 128], [128, RS]]))
                  xq = []
                  for k in range(3):
                      xk = gpts.tile([128, RS], F32)
                      nc.sync.dma_start(xk[:], t["xq%d" % k][:])
                      xq.append(xk)
                  S_ap = gt[:, 3:4, :].rearrange("p a b -> p (a b)")
                  n2 = gpts.tile([128, RS], F32)
                  sq = gpts.tile([128, RS], F32)
                  gk = gpts.tile([128, RS], F32)
                  for k in range(3):
                      hy_ap = gt[:, k:k + 1, :].rearrange("p a b -> p (a b)")
                      nc.vector.tensor_tensor(out=gk[:], in0=xq[k][:], in1=S_ap,
                                              op=ALU.mult)
                      nc.vector.tensor_tensor(out=gk[:], in0=gk[:], in1=hy_ap,
                                              op=ALU.subtract)
                      nc.scalar.activation(sq[:], gk[:], AF.Square)
                      if k == 0:
                          nc.vector.tensor_copy(out=n2[:], in_=sq[:])
                      else:
                          nc.vector.tensor_tensor(out=n2[:], in0=n2[:], in1=sq[:],
                                                  op=ALU.add)
                  # ||g|| = exp(0.5 ln n2); dev = ||g|| - 1; |dev| -> out cols 32:40
                  nc.scalar.activation(sq[:], n2[:], AF.Ln)
                  nc.scalar.activation(n2[:], sq[:], AF.Exp, scale=0.5)
                  nc.vector.tensor_scalar(out=sq[:], in0=n2[:], scalar1=1.0,
                                          scalar2=None, op0=ALU.subtract)
                  nc.scalar.activation(outsb[:, 32:40], sq[:], AF.Abs)

            nc.sync.dma_start(out[:], outsb[:])

    nc.compile()
    return nc


def _get_nc():
    if "nc" not in _CACHE:
        _CACHE["nc"] = _build()
    return _CACHE["nc"]


def _pack_core_inputs(inputs):
    """Slice/transform full inputs into 8 per-core input maps (+host context)."""
    in_maps = []
    host = []
    flat = {k: np.ascontiguousarray(inputs[k]).reshape(B * P)
            for k in ["sdf_pred", "sdf_target", "uncertainty",
                      "occupancy_pred", "occupancy_target"]}
    nrm_a = np.ascontiguousarray(inputs["normals_pred"]).reshape(B * P, 3)
    nrm_b = np.ascontiguousarray(inputs["normals_target"]).reshape(B * P, 3)
    pcp = np.asarray(inputs["point_cloud_pred"], dtype=np.float32)
    pct = np.asarray(inputs["point_cloud_target"], dtype=np.float32)

    for c in range(8):
        b, q = c // 4, c % 4
        lo = c * (B * P // 8)
        hi = lo + B * P // 8
        m = {}
        m["sdfa"] = flat["sdf_pred"][lo:hi].reshape(EWP, EWF)
        m["sdfb"] = flat["sdf_target"][lo:hi].reshape(EWP, EWF)
        m["uncw"] = flat["uncertainty"][lo:hi].reshape(EWP, EWF)
        m["occp"] = flat["occupancy_pred"][lo:hi].reshape(EWP, EWF)
        m["occt"] = flat["occupancy_target"][lo:hi].reshape(EWP, EWF)
        for k, nm in enumerate(["nax", "nay", "naz"]):
            m[nm] = np.ascontiguousarray(nrm_a[lo:hi, k]).reshape(EWP, EWF)
        for k, nm in enumerate(["nbx", "nby", "nbz"]):
            m[nm] = np.ascontiguousarray(nrm_b[lo:hi, k]).reshape(EWP, EWF)

        X = pcp[b]          # [N,3]
        Y = pct[b]          # [M,3]
        xx = (X * X).sum(1).astype(np.float32)
        yy = (Y * Y).sum(1).astype(np.float32)
        rows = slice(q * NQ, (q + 1) * NQ)
        Xq = X[rows]

        m["XTq"] = np.ascontiguousarray(
            np.vstack([-2.0 * Xq.T, np.ones((1, NQ), np.float32)]))
        m["YTm"] = np.ascontiguousarray(np.vstack([Y.T, yy[None, :]]))
        Yq = Y[rows]
        m["YTnq"] = np.ascontiguousarray(
            np.vstack([-2.0 * Yq.T, np.ones((1, NQ), np.float32)]))
        m["YTn"] = np.ascontiguousarray(
            np.vstack([-2.0 * Y.T, np.ones((1, M), np.float32)]))
        m["XTm"] = np.ascontiguousarray(np.vstack([X.T, xx[None, :]]))
        m["XTg"] = np.ascontiguousarray(m["XTm"][:, rows])
        m["xxqeT"] = np.ascontiguousarray(xx[rows].reshape(RS, 128).T)
        m["yyqeT"] = np.ascontiguousarray(yy[rows].reshape(RS, 128).T)
        m["yyfeT"] = np.ascontiguousarray(yy.reshape(GS, 128).T)
        y4 = np.concatenate([Y.reshape(GS, 128, 3),
                             np.ones((GS, 128, 1), np.float32)], axis=2)
        # Y4g[p, 4s+k] = y4[s, p, k]
        m["Y4g"] = _to_bf16(np.ascontiguousarray(
            y4.transpose(1, 0, 2).reshape(128, 4 * GS)))
        for k in range(3):
            m["xq%d" % k] = np.ascontiguousarray(
                Xq[:, k].reshape(RS, 128).T)
        in_maps.append(m)
        host.append(dict(xxq=xx[rows].reshape(RS, 128),
                         yyq=yy[rows].reshape(RS, 128)))
    return in_maps, host


def _to_bf16(a):
    import ml_dtypes
    return a.astype(ml_dtypes.bfloat16)


def _combine(results, host):
    """Host-side combine of per-core [128,64] partial tiles (float64)."""
    s_l1 = s_b1 = s_b2 = s_cos = s_unc = 0.0
    s_rowmin = s_colmin = s_A = s_C = s_gp = 0.0
    for c in range(8):
        o = results[c]["out"].astype(np.float64)
        # chamfer mins arrive clamped with xx/yy included; sqrt in f64
        s_rowmin += np.sqrt(np.maximum(o[:, 0:RS], 1e-12)).sum()
        s_colmin += np.sqrt(np.maximum(o[:, 16:16 + RS], 1e-12)).sum()
        s_A += o[:, 8:16].sum()
        s_C += o[:, 24:32].sum()
        s_gp += o[:, 32:40].sum()
        s_l1 += o[0:EWP, 40].sum()
        s_b1 += o[0:EWP, 41].sum()
        s_b2 += o[0:EWP, 42].sum()
        s_cos += o[0:EWP, 43].sum()
        s_unc += o[0:EWP, 44].sum()

    sdf_loss = s_l1 / (B * P)
    bce = -(s_b1 + s_b2) / (B * P)
    p_t = np.exp(-bce)
    occ_loss = FOCAL_ALPHA * (1.0 - p_t) ** FOCAL_GAMMA * bce
    normal_loss = 1.0 - s_cos / (B * P)
    unc_reg = s_unc / (B * P)
    chamfer = s_rowmin / (B * N) + s_colmin / (B * M)
    emd = (s_A / N + s_C / M) / B
    gp = s_gp / (B * N)
    total = (W_SDF * sdf_loss + W_OCC * occ_loss + W_NORMAL * normal_loss
             + W_CHAMFER * chamfer + W_EMD * emd + W_EMD_GP * gp
             + W_UNC * unc_reg)
    return np.float32(total)


def run(inputs, trace=False):
    nc = _get_nc()
    in_maps, host = _pack_core_inputs(inputs)
    res = run_bass_kernel_spmd(nc, in_maps, list(range(8)), trace=trace)
    total = _combine(res.results, host)
    return total, res


def kernel(**inputs) -> np.ndarray:
    total, _ = run(inputs)
    return np.asarray(total, dtype=np.float32)



# revision 8
# speedup vs baseline: 2.4159x; 2.0848x over previous
"""Trainium2 Bass kernel for nn_AdvancedLossFunction (8-core SPMD).

Sharding: 8 cores = 2 batches x 4 n-quarters. Core c handles batch b=c//4,
n-quarter q=c%4 (1024 of the 4096 pred points) x ALL m target points, in
COLUMN orientation only (m on partitions, 32 stripes of [128, 1024]), plus
1/8 of the elementwise (P=100000) losses.

Math (validated vs jax reference in numpy, exact-equivalent):
  unshifted softmax E = exp(-d/tau) everywhere (fp32 sums stay normal:
  max 50*min_dist ~ 75 < 87):
    s_n = sum_m E, t_n = sum_m E*d  -> PE ones-contraction over partitions
    u_m = sum_n E (ACT accum), v_m = sum_n E*d (stt accum); AllReduce-add
    A = t/s, C = v/u;  w1r=(1+50A)/(BN*s), w2r=50/(BN*s), sim. w1c/w2c
  grad: H = E*(qv*w1r[n] - w2r[n]) + E*(qv*w1c[m] - w2c[m]), qv = 1/d
    (HY)_k,S via TWO streams (ec, ec*qv) x stacked lhsT [Y4|Y4*w1c|Y4*w2c]
    -> psum accum over stripes; w1r/w2r applied POST-contraction per n.
Device strategy:
  - d^2 via k=11 bf16 hi/lo-split matmul (xh*yh+xl*yh+xh*yl+xxh+xxl),
    yy as fp32 ACT bias; ~2e-5 abs err (fp32-like for this data)
  - d = d2 * AbsRsqrt(d2+1e-6): Copy+AbsRsqrt live in ONE table set,
    |.| clamps negative d2 (exact-dup points) without extra ops
  - ACT tables pinned to {abs_reciprocal_sqrt_and_small,
    natural_log_exp_and_others}: 2 loads total (was 91 in the old design)
  - chamfer mins: col-min DVE reduce per stripe; row-min = elementwise
    TT-min across stripes then gpsimd partition max of negated
  - each core outputs [128,64] partials; host combines in f64
"""
import numpy as np
from contextlib import ExitStack

import concourse.bass as bass
import concourse.bass_isa as bass_isa
import concourse.bacc as bacc
import concourse.tile as tile
from concourse import mybir
from concourse.bass_utils import run_bass_kernel_spmd

F32 = mybir.dt.float32
BF16 = mybir.dt.bfloat16
AF = mybir.ActivationFunctionType
ALU = mybir.AluOpType

TAU = 0.02
INV_TAU = 50.0
B, P = 2, 100000
N = M = 4096
NQ = 1024          # n-quarter size per core
RS = NQ // 128     # 8 sub-blocks of the quarter (layout n = i*128 + p)
GS = M // 128      # 32 column stripes (all m)
KD = 11            # hi/lo split matmul contraction depth
EPS = 1e-6
EWP, EWF = 125, 200  # per-core elementwise slice 25000 = 125*200

FOCAL_GAMMA, FOCAL_ALPHA = 2.0, 0.75
W_SDF, W_OCC, W_NORMAL, W_CHAMFER, W_EMD, W_EMD_GP, W_UNC = \
    1.0, 1.0, 0.1, 1.0, 0.25, 0.05, 0.1

_CACHE = {}


class _PinnedBacc(bacc.Bacc):
    """Pin activations to two table sets so the compiler never thrashes:
    abs_reciprocal_sqrt_and_small (AbsRsqrt+Copy) for the d/qv phase and
    natural_log_exp_and_others (Exp/Ln/Abs/Square/Copy) for the rest."""

    _PIN = ("abs_reciprocal_sqrt_and_small", "natural_log_exp_and_others")

    def insert_act_table_loads(self):
        has_activation = any(
            isinstance(i, mybir.InstActivation)
            for b_ in self.main_func.blocks
            for i in b_.instructions
        )
        if not has_activation:
            return
        from concourse.hw_specs import get_activation_tables
        import bass_rust as _br

        tables = []
        for name, funcs in get_activation_tables(self.m.arch).items():
            tables.append((name, funcs if name in self._PIN else set()))
        _br.insert_act_table_loads(self, tables)


def _declare_inputs(nc):
    t = {}
    for name in ["sdfa", "sdfb", "uncw", "occp", "occt",
                 "nax", "nay", "naz", "nbx", "nby", "nbz"]:
        t[name] = nc.dram_tensor(name, [EWP, EWF], F32, kind="ExternalInput")
    t["YT11"] = nc.dram_tensor("YT11", [KD, M], BF16, kind="ExternalInput")
    t["XT11"] = nc.dram_tensor("XT11", [KD, NQ], BF16, kind="ExternalInput")
    t["yyfeT"] = nc.dram_tensor("yyfeT", [128, GS], F32, kind="ExternalInput")
    t["Y4g"] = nc.dram_tensor("Y4g", [128, 4 * GS], BF16, kind="ExternalInput")
    t["xq0"] = nc.dram_tensor("xq0", [128, RS], F32, kind="ExternalInput")
    t["xq1"] = nc.dram_tensor("xq1", [128, RS], F32, kind="ExternalInput")
    t["xq2"] = nc.dram_tensor("xq2", [128, RS], F32, kind="ExternalInput")
    return t


def _ew_stage(nc, t, outsb, pool):
    """Elementwise losses on the core's 25000-point slice -> outsb cols 40-44."""
    def load(name):
        s = pool.tile([EWP, EWF], F32, tag="ewin_" + name)
        nc.sync.dma_start(s[:], t[name][:])
        return s

    sdfa, sdfb = load("sdfa"), load("sdfb")
    diff = pool.tile([EWP, EWF], F32, tag="ewt0")
    nc.vector.tensor_tensor(out=diff[:], in0=sdfa[:], in1=sdfb[:], op=ALU.subtract)
    junk = pool.tile([EWP, EWF], F32, tag="ewt1")
    nc.scalar.activation(junk[:], diff[:], AF.Abs, accum_out=outsb[0:EWP, 40:41])

    occp, occt = load("occp"), load("occt")
    lnp = pool.tile([EWP, EWF], F32, tag="ewt2")
    nc.scalar.activation(lnp[:], occp[:], AF.Ln)
    j2 = pool.tile([EWP, EWF], F32, tag="ewt3")
    nc.vector.tensor_tensor(out=j2[:], in0=occt[:], in1=lnp[:], op=ALU.mult)
    jr = pool.tile([EWP, EWF], F32, tag="ewtr")
    nc.vector.tensor_scalar(out=jr[:], in0=j2[:], scalar1=1.0, scalar2=None,
                            op0=ALU.mult, op1=ALU.add, accum_out=outsb[0:EWP, 41:42])
    onemp = pool.tile([EWP, EWF], F32, tag="ewt4")
    nc.vector.tensor_scalar(out=onemp[:], in0=occp[:], scalar1=-1.0, scalar2=1.0,
                            op0=ALU.mult, op1=ALU.add)
    ln1mp = pool.tile([EWP, EWF], F32, tag="ewt5")
    nc.scalar.activation(ln1mp[:], onemp[:], AF.Ln)
    onemt = pool.tile([EWP, EWF], F32, tag="ewt6")
    nc.vector.tensor_scalar(out=onemt[:], in0=occt[:], scalar1=-1.0, scalar2=1.0,
                            op0=ALU.mult, op1=ALU.add)
    nc.vector.tensor_tensor(out=lnp[:], in0=onemt[:], in1=ln1mp[:], op=ALU.mult)
    nc.vector.tensor_scalar(out=jr[:], in0=lnp[:], scalar1=1.0, scalar2=None,
                            op0=ALU.mult, op1=ALU.add, accum_out=outsb[0:EWP, 42:43])

    nax, nay, naz = load("nax"), load("nay"), load("naz")
    nbx, nby, nbz = load("nbx"), load("nby"), load("nbz")
    dot = pool.tile([EWP, EWF], F32, tag="ewt0")
    nc.vector.tensor_tensor(out=dot[:], in0=nax[:], in1=nbx[:], op=ALU.mult)
    tmp = pool.tile([EWP, EWF], F32, tag="ewt1")
    nc.vector.tensor_tensor(out=tmp[:], in0=nay[:], in1=nby[:], op=ALU.mult)
    nc.vector.tensor_tensor(out=dot[:], in0=dot[:], in1=tmp[:], op=ALU.add)
    nc.vector.tensor_tensor(out=tmp[:], in0=naz[:], in1=nbz[:], op=ALU.mult)
    nc.vector.tensor_tensor(out=dot[:], in0=dot[:], in1=tmp[:], op=ALU.add)

    def rnorm(cx, cy, cz, tag):
        n2 = pool.tile([EWP, EWF], F32, tag=tag)
        s1 = pool.tile([EWP, EWF], F32, tag=tag + "s")
        nc.scalar.activation(n2[:], cx[:], AF.Square)
        nc.scalar.activation(s1[:], cy[:], AF.Square)
        nc.vector.tensor_tensor(out=n2[:], in0=n2[:], in1=s1[:], op=ALU.add)
        nc.scalar.activation(s1[:], cz[:], AF.Square)
        nc.vector.tensor_tensor(out=n2[:], in0=n2[:], in1=s1[:], op=ALU.add)
        # 1/norm = exp(-0.5*ln(n2))
        nc.scalar.activation(s1[:], n2[:], AF.Ln)
        nc.scalar.activation(n2[:], s1[:], AF.Exp, scale=-0.5)
        return n2

    ra = rnorm(nax, nay, naz, "ewt2")
    rb = rnorm(nbx, nby, nbz, "ewt4")
    nc.vector.tensor_tensor(out=dot[:], in0=dot[:], in1=ra[:], op=ALU.mult)
    nc.vector.tensor_tensor(out=dot[:], in0=dot[:], in1=rb[:], op=ALU.mult)
    nc.vector.tensor_scalar(out=tmp[:], in0=dot[:], scalar1=1.0, scalar2=None,
                            op0=ALU.mult, op1=ALU.add, accum_out=outsb[0:EWP, 43:44])

    uncw = load("uncw")
    onemu = pool.tile([EWP, EWF], F32, tag="ewt0")
    nc.vector.tensor_scalar(out=onemu[:], in0=uncw[:], scalar1=-1.0, scalar2=1.0,
                            op0=ALU.mult, op1=ALU.add)
    nc.vector.tensor_tensor(out=onemu[:], in0=uncw[:], in1=onemu[:], op=ALU.mult)
    nc.vector.tensor_scalar(out=tmp[:], in0=onemu[:], scalar1=1.0, scalar2=None,
                            op0=ALU.mult, op1=ALU.add, accum_out=outsb[0:EWP, 44:45])


def _build(trn_type="TRN2"):
    nc = _PinnedBacc(trn_type, target_bir_lowering=False)
    t = _declare_inputs(nc)
    out = nc.dram_tensor("out", [128, 64], F32, kind="ExternalOutput")

    with tile.TileContext(nc) as tc:
        with ExitStack() as ctx:
            persist = ctx.enter_context(tc.tile_pool(name="persist", bufs=1))
            dram = ctx.enter_context(tc.tile_pool(name="dram", bufs=1, space="DRAM"))
            # big per-element stores (64KB/partition each)
            qepool = ctx.enter_context(tc.tile_pool(name="qe", bufs=1))

            outsb = persist.tile([128, 64], F32)
            nc.vector.memset(outsb[:], 0.0)
            u = persist.tile([128, GS], F32)
            v = persist.tile([128, GS], F32)
            racc = persist.tile([128, NQ], BF16)
            nc.vector.memset(racc[:], 1.0e30)
            Y4g = persist.tile([128, 4 * GS], BF16)
            nc.sync.dma_start(Y4g[:], t["Y4g"][:])
            yyfeT = persist.tile([128, GS], F32)
            nc.sync.dma_start(yyfeT[:], t["yyfeT"][:])

            epsb = persist.tile([128, 1], F32)
            nc.vector.memset(epsb[:], EPS)
            qvall = qepool.tile([128, GS * NQ], BF16)   # 1/d
            ecall = qepool.tile([128, GS * NQ], BF16)   # exp(-50 d)

            dctx = ExitStack()
            dpool = dctx.enter_context(tc.tile_pool(name="dp", bufs=1))
            dall = dpool.tile([128, GS * NQ], BF16)     # d2 -> d in place

            # ---- phase S: d^2 matmuls, d2->bf16, qv=AbsRsqrt, d=d2*qv ----
            with ExitStack() as sctx:
                mpool = sctx.enter_context(tc.tile_pool(name="mp", bufs=1))
                ppool = sctx.enter_context(
                    tc.tile_pool(name="pp", bufs=2, space="PSUM"))
                YT11 = mpool.tile([KD, M], BF16)
                nc.sync.dma_start(YT11[:], t["YT11"][:])
                XT11 = mpool.tile([KD, NQ], BF16)
                nc.sync.dma_start(XT11[:], t["XT11"][:])

                for s in range(GS):
                    lhsT = YT11[:, s * 128:(s + 1) * 128]
                    pd2 = ppool.tile([128, NQ], F32, tag="pd2")
                    for h in range(2):
                        nc.tensor.matmul(pd2[:, h * 512:(h + 1) * 512], lhsT,
                                         XT11[:, h * 512:(h + 1) * 512],
                                         start=True, stop=True)
                    dsl = dall[:, s * NQ:(s + 1) * NQ]
                    qsl = qvall[:, s * NQ:(s + 1) * NQ]
                    # d2 (+yy bias) -> bf16; Copy is in the absrsqrt table set
                    nc.scalar.activation(dsl, pd2[:], AF.Identity,
                                         bias=yyfeT[:, s:s + 1], scale=1.0)
                    # qv = 1/sqrt(|d2 + eps|): |.| clamps matmul-noise negatives
                    nc.scalar.activation(qsl, dsl, AF.Abs_reciprocal_sqrt,
                                         bias=epsb[:], scale=1.0)
                    # d = d2 * qv  (in place over d2)
                    nc.vector.tensor_tensor(out=dsl, in0=dsl, in1=qsl,
                                            op=ALU.mult)

            # ---- phase E: ec=exp(-50d), u/v accums, s/t PE sums, mins ----
            with ExitStack() as ectx:
                epool = ectx.enter_context(tc.tile_pool(name="ep", bufs=2))
                stp = ctx.enter_context(
                    tc.tile_pool(name="stp", bufs=1, space="PSUM"))
                ones1 = persist.tile([128, 1], BF16)
                nc.vector.memset(ones1[:], 1.0)
                s_ps = stp.tile([1, NQ], F32)
                t_ps = stp.tile([1, NQ], F32)
                for s in range(GS):
                    dsl = dall[:, s * NQ:(s + 1) * NQ]
                    esl = ecall[:, s * NQ:(s + 1) * NQ]
                    nc.scalar.activation(esl, dsl, AF.Exp, scale=-INV_TAU,
                                         accum_out=u[:, s:s + 1])
                    dec = epool.tile([128, NQ], BF16, tag="dec")
                    nc.vector.scalar_tensor_tensor(
                        out=dec[:], in0=esl, scalar=1.0, in1=dsl,
                        op0=ALU.mult, op1=ALU.mult, accum_out=v[:, s:s + 1])
                    # col min (chamfer partial) + row min accumulation
                    nc.vector.tensor_reduce(out=outsb[:, 8 + s:9 + s], in_=dsl,
                                            axis=mybir.AxisListType.X, op=ALU.min)
                    nc.vector.tensor_tensor(out=racc[:], in0=racc[:], in1=dsl,
                                            op=ALU.min)
                    for h in range(2):
                        hs = slice(h * 512, (h + 1) * 512)
                        nc.tensor.matmul(s_ps[:, hs], ones1[:], esl[:, hs],
                                         start=(s == 0), stop=(s == GS - 1),
                                         skip_group_check=True)
                        nc.tensor.matmul(t_ps[:, hs], ones1[:], dec[:, hs],
                                         start=(s == 0), stop=(s == GS - 1),
                                         skip_group_check=True)

            dctx.close()  # free dall (64KB/p) before the post-phase pools

            with ExitStack() as ectx:
                epool = ectx.enter_context(tc.tile_pool(name="po", bufs=1))
                # ---- u/v AllReduce over the 4 cores of this batch ----
                gin = dram.tile([1, 2 * M], F32)
                gout = dram.tile([1, 2 * M], F32)
                def dump_mmajor(sb_tile, off):
                    ap = bass.AP(tensor=gin.tensor, offset=gin.offset + off,
                                 ap=[[1, 128], [128, GS]])
                    nc.sync.dma_start(out=ap, in_=sb_tile[:])
                dump_mmajor(u, 0)
                dump_mmajor(v, M)
                nc.gpsimd.collective_compute(
                    "AllReduce", ALU.add,
                    replica_groups=[[0, 1, 2, 3], [4, 5, 6, 7]],
                    ins=[gin[:]], outs=[gout[:]])
                ug = persist.tile([128, GS], F32)
                vg = persist.tile([128, GS], F32)
                for tt_, off in ((ug, 0), (vg, M)):
                    nc.sync.dma_start(
                        out=tt_[:],
                        in_=bass.AP(tensor=gout.tensor, offset=gout.offset + off,
                                    ap=[[1, 128], [128, GS]]))

                # ---- EW losses (overlap the collective) ----
                _ew_stage(nc, t, outsb, epool)

                # ---- row stats finalize: s,t -> A, w1r, w2r in [128,8] ----
                s_sb = epool.tile([1, NQ], F32, tag="ssb")
                nc.scalar.copy(s_sb[:], s_ps[:])
                t_sb = epool.tile([1, NQ], F32, tag="tsb")
                nc.scalar.copy(t_sb[:], t_ps[:])
                std = dram.tile([1, 2 * NQ], F32)
                nc.sync.dma_start(
                    out=bass.AP(tensor=std.tensor, offset=std.offset,
                                ap=[[1, NQ]]), in_=s_sb[:])
                nc.sync.dma_start(
                    out=bass.AP(tensor=std.tensor, offset=std.offset + NQ,
                                ap=[[1, NQ]]), in_=t_sb[:])
                s8 = persist.tile([128, RS], F32)
                t8 = persist.tile([128, RS], F32)
                for tt_, off in ((s8, 0), (t8, NQ)):
                    nc.sync.dma_start(
                        out=tt_[:],
                        in_=bass.AP(tensor=std.tensor, offset=std.offset + off,
                                    ap=[[1, 128], [128, RS]]))

                # row min: partition max of negated racc
                nneg = epool.tile([128, NQ], BF16, tag="nneg")
                nc.vector.tensor_scalar(out=nneg[:], in0=racc[:], scalar1=-1.0,
                                        scalar2=None, op0=ALU.mult)
                pmax = epool.tile([128, NQ], F32, tag="pmax")
                nc.gpsimd.partition_all_reduce(pmax[:], nneg[:], 128,
                                               bass_isa.ReduceOp.max)
                m1d = dram.tile([1, NQ], F32)
                nc.sync.dma_start(out=bass.AP(tensor=m1d.tensor, offset=m1d.offset,
                                              ap=[[1, NQ]]), in_=pmax[0:1, :])
                m1q = epool.tile([128, RS], F32, tag="m1q")
                nc.sync.dma_start(
                    out=m1q[:],
                    in_=bass.AP(tensor=m1d.tensor, offset=m1d.offset,
                                ap=[[1, 128], [128, RS]]))
                nc.vector.tensor_scalar(out=outsb[:, 54:62], in0=m1q[:],
                                        scalar1=-1.0, scalar2=None, op0=ALU.mult)

            # ---- stats -> coefficients ----
            fin = ctx.enter_context(tc.tile_pool(name="fin", bufs=1))
            urec = fin.tile([128, GS], F32)
            nc.vector.reciprocal(out=urec[:], in_=ug[:])
            Cc = fin.tile([128, GS], F32)
            nc.vector.tensor_tensor(out=Cc[:], in0=vg[:], in1=urec[:], op=ALU.mult)
            csc = fin.tile([128, GS], F32)
            nc.vector.tensor_scalar(out=csc[:], in0=Cc[:], scalar1=1.0,
                                    scalar2=None, op0=ALU.mult, op1=ALU.add,
                                    accum_out=outsb[:, 45:46])
            w1c = fin.tile([128, GS], F32)
            nc.vector.tensor_scalar(out=csc[:], in0=Cc[:], scalar1=INV_TAU,
                                    scalar2=1.0, op0=ALU.mult, op1=ALU.add)
            nc.vector.tensor_tensor(out=csc[:], in0=csc[:], in1=urec[:],
                                    op=ALU.mult)
            nc.vector.tensor_scalar(out=w1c[:], in0=csc[:],
                                    scalar1=1.0 / (B * M), scalar2=None,
                                    op0=ALU.mult)
            w2c = fin.tile([128, GS], F32)
            nc.vector.tensor_scalar(out=w2c[:], in0=urec[:],
                                    scalar1=INV_TAU / (B * M), scalar2=None,
                                    op0=ALU.mult)

            srec = fin.tile([128, RS], F32)
            nc.vector.reciprocal(out=srec[:], in_=s8[:])
            A8 = fin.tile([128, RS], F32)
            nc.vector.tensor_tensor(out=A8[:], in0=t8[:], in1=srec[:], op=ALU.mult)
            nc.vector.tensor_copy(out=outsb[:, 46:54], in_=A8[:])
            w1r = fin.tile([128, RS], F32)
            nc.vector.tensor_scalar(out=w1r[:], in0=A8[:], scalar1=INV_TAU,
                                    scalar2=1.0, op0=ALU.mult, op1=ALU.add)
            nc.vector.tensor_tensor(out=w1r[:], in0=w1r[:], in1=srec[:],
                                    op=ALU.mult)
            nc.vector.tensor_scalar(out=w1r[:], in0=w1r[:],
                                    scalar1=1.0 / (B * N), scalar2=None,
                                    op0=ALU.mult)
            w2r = fin.tile([128, RS], F32)
            nc.vector.tensor_scalar(out=w2r[:], in0=srec[:],
                                    scalar1=INV_TAU / (B * N), scalar2=None,
                                    op0=ALU.mult)

            # ---- phase P3: gradient contractions ----
            with ExitStack() as gctx:
                gpool = gctx.enter_context(tc.tile_pool(name="gp", bufs=2))
                gps = gctx.enter_context(
                    tc.tile_pool(name="gps", bufs=1, space="PSUM"))
                pacc1 = gps.tile([12, NQ], F32)
                pacc2 = gps.tile([12, NQ], F32)
                for s in range(GS):
                    esl = ecall[:, s * NQ:(s + 1) * NQ]
                    qsl = qvall[:, s * NQ:(s + 1) * NQ]
                    L = gpool.tile([128, 12], BF16, tag="L")
                    y4s = Y4g[:, 4 * s:4 * s + 4]
                    nc.vector.tensor_copy(out=L[:, 0:4], in_=y4s)
                    nc.vector.tensor_scalar(out=L[:, 4:8], in0=y4s,
                                            scalar1=w1c[:, s:s + 1],
                                            scalar2=None, op0=ALU.mult)
                    nc.vector.tensor_scalar(out=L[:, 8:12], in0=y4s,
                                            scalar1=w2c[:, s:s + 1],
                                            scalar2=None, op0=ALU.mult)
                    eq = gpool.tile([128, NQ], BF16, tag="eq")
                    nc.vector.tensor_tensor(out=eq[:], in0=esl, in1=qsl,
                                            op=ALU.mult)
                    for h in range(2):
                        hs = slice(h * 512, (h + 1) * 512)
                        nc.tensor.matmul(pacc1[:, hs], L[:], eq[:, hs],
                                         start=(s == 0), stop=(s == GS - 1),
                                         skip_group_check=True)
                        nc.tensor.matmul(pacc2[:, hs], L[:], esl[:, hs],
                                         start=(s == 0), stop=(s == GS - 1),
                                         skip_group_check=True)

                # evacuate pacc to DRAM, regroup to [128, 24, 8]
                gsb1 = gpool.tile([12, NQ], F32, tag="gsb1")
                nc.scalar.copy(gsb1[:], pacc1[:])
                gsb2 = gpool.tile([12, NQ], F32, tag="gsb2")
                nc.scalar.copy(gsb2[:], pacc2[:])
                gd = dram.tile([1, 24 * NQ], F32)
                nc.sync.dma_start(
                    out=bass.AP(tensor=gd.tensor, offset=gd.offset,
                                ap=[[NQ, 12], [1, NQ]]),
                    in_=gsb1[:])
                nc.sync.dma_start(
                    out=bass.AP(tensor=gd.tensor, offset=gd.offset + 12 * NQ,
                                ap=[[NQ, 12], [1, NQ]]),
                    in_=gsb2[:])
                gt = gpool.tile([128, 24, RS], F32, tag="gt")
                nc.sync.dma_start(
                    out=gt[:],
                    in_=bass.AP(tensor=gd.tensor, offset=gd.offset,
                                ap=[[1, 128], [NQ, 24], [128, RS]]))

                def row(r):
                    return gt[:, r:r + 1, :].rearrange("p a b -> p (a b)")

                xq = []
                for k in range(3):
                    xk = gpool.tile([128, RS], F32, tag="xq%d" % k)
                    nc.sync.dma_start(xk[:], t["xq%d" % k][:])
                    xq.append(xk)

                # HY_k = w1r*P1'_k - w2r*P2'_k + P3_k - P4_k   (k=3 -> S)
                tA = gpool.tile([128, RS], F32, tag="tA")
                tBt = gpool.tile([128, RS], F32, tag="tB")
                n2 = gpool.tile([128, RS], F32, tag="n2")
                Srow = gpool.tile([128, RS], F32, tag="Srow")
                for k in range(4):
                    dst = Srow if k == 3 else tA
                    nc.vector.tensor_tensor(out=tBt[:], in0=w1r[:], in1=row(k),
                                            op=ALU.mult)
                    nc.vector.tensor_tensor(out=dst[:], in0=w2r[:],
                                            in1=row(12 + k), op=ALU.mult)
                    nc.vector.tensor_tensor(out=tBt[:], in0=tBt[:], in1=dst[:],
                                            op=ALU.subtract)
                    nc.vector.tensor_tensor(out=tBt[:], in0=tBt[:], in1=row(4 + k),
                                            op=ALU.add)
                    nc.vector.tensor_tensor(out=dst[:], in0=tBt[:],
                                            in1=row(20 + k), op=ALU.subtract)
                    if k == 3:
                        break
                    # g_k = x_k*S - HY_k ... but S (k=3) comes last; stash HY_k
                    nc.vector.tensor_copy(out=gt[:, k:k + 1, :].rearrange(
                        "p a b -> p (a b)"), in_=dst[:])
                for k in range(3):
                    nc.vector.tensor_tensor(out=tA[:], in0=xq[k][:], in1=Srow[:],
                                            op=ALU.mult)
                    nc.vector.tensor_tensor(out=tA[:], in0=tA[:], in1=row(k),
                                            op=ALU.subtract)
                    nc.scalar.activation(tBt[:], tA[:], AF.Square)
                    if k == 0:
                        nc.vector.tensor_copy(out=n2[:], in_=tBt[:])
                    else:
                        nc.vector.tensor_tensor(out=n2[:], in0=n2[:], in1=tBt[:],
                                                op=ALU.add)
                # ||g|| = exp(0.5 ln n2); |.-1| -> outsb[:, 0:8]
                nc.scalar.activation(tBt[:], n2[:], AF.Ln)
                nc.scalar.activation(n2[:], tBt[:], AF.Exp, scale=0.5)
                nc.vector.tensor_scalar(out=tA[:], in0=n2[:], scalar1=1.0,
                                        scalar2=None, op0=ALU.subtract)
                nc.scalar.activation(outsb[:, 0:8], tA[:], AF.Abs)

            nc.sync.dma_start(out[:], outsb[:])

    nc.compile()
    return nc


def _get_nc():
    if "nc" not in _CACHE:
        _CACHE["nc"] = _build()
    return _CACHE["nc"]


def _to_bf16(a):
    import ml_dtypes
    return np.ascontiguousarray(a).astype(ml_dtypes.bfloat16)


def _pack_core_inputs(inputs):
    """Slice/transform full inputs into 8 per-core input maps."""
    import ml_dtypes
    in_maps = []
    flat = {k: np.ascontiguousarray(inputs[k]).reshape(B * P)
            for k in ["sdf_pred", "sdf_target", "uncertainty",
                      "occupancy_pred", "occupancy_target"]}
    nrm_a = np.ascontiguousarray(inputs["normals_pred"]).reshape(B * P, 3)
    nrm_b = np.ascontiguousarray(inputs["normals_target"]).reshape(B * P, 3)
    pcp = np.asarray(inputs["point_cloud_pred"], dtype=np.float32)
    pct = np.asarray(inputs["point_cloud_target"], dtype=np.float32)

    bf = lambda a: a.astype(ml_dtypes.bfloat16).astype(np.float32)

    for c in range(8):
        b, q = c // 4, c % 4
        lo = c * (B * P // 8)
        hi = lo + B * P // 8
        m = {}
        m["sdfa"] = flat["sdf_pred"][lo:hi].reshape(EWP, EWF)
        m["sdfb"] = flat["sdf_target"][lo:hi].reshape(EWP, EWF)
        m["uncw"] = flat["uncertainty"][lo:hi].reshape(EWP, EWF)
        m["occp"] = flat["occupancy_pred"][lo:hi].reshape(EWP, EWF)
        m["occt"] = flat["occupancy_target"][lo:hi].reshape(EWP, EWF)
        for k, nm in enumerate(["nax", "nay", "naz"]):
            m[nm] = np.ascontiguousarray(nrm_a[lo:hi, k]).reshape(EWP, EWF)
        for k, nm in enumerate(["nbx", "nby", "nbz"]):
            m[nm] = np.ascontiguousarray(nrm_b[lo:hi, k]).reshape(EWP, EWF)

        X = pcp[b]          # [N,3]
        Y = pct[b]          # [M,3]
        rows = slice(q * NQ, (q + 1) * NQ)
        Xq = X[rows]
        xx = (X * X).sum(1).astype(np.float32)[rows]
        yy = (Y * Y).sum(1).astype(np.float32)

        Xh = bf(Xq); Xl = Xq - Xh
        Yh = bf(Y); Yl = Y - Yh
        xxh = bf(xx); xxl = xx - xxh
        # pairing: (-2yh,xh)x3, (-2yl,xh)x3, (-2yh,xl)x3, (1,xxh), (1,xxl)
        yt = np.vstack([-2.0 * Yh.T, -2.0 * Yl.T, -2.0 * Yh.T,
                        np.ones((2, M), np.float32)])
        xt = np.vstack([Xh.T, Xh.T, Xl.T, xxh[None, :], xxl[None, :]])
        m["YT11"] = _to_bf16(yt)
        m["XT11"] = _to_bf16(xt)
        m["yyfeT"] = np.ascontiguousarray(yy.reshape(GS, 128).T)
        y4 = np.concatenate([Y.reshape(GS, 128, 3),
                             np.ones((GS, 128, 1), np.float32)], axis=2)
        m["Y4g"] = _to_bf16(y4.transpose(1, 0, 2).reshape(128, 4 * GS))
        for k in range(3):
            m["xq%d" % k] = np.ascontiguousarray(Xq[:, k].reshape(RS, 128).T)
        in_maps.append(m)
    return in_maps


def _combine(results):
    """Host-side combine of per-core [128,64] partial tiles (float64)."""
    s_l1 = s_b1 = s_b2 = s_cos = s_unc = 0.0
    s_m1 = s_A = s_gp = 0.0
    s_C = 0.0
    m2 = [None, None]
    for c in range(8):
        o = results[c]["out"].astype(np.float64)
        b = c // 4
        s_gp += o[:, 0:8].sum()
        m2c = o[:, 8:40]
        m2[b] = m2c if m2[b] is None else np.minimum(m2[b], m2c)
        s_l1 += o[0:EWP, 40].sum()
        s_b1 += o[0:EWP, 41].sum()
        s_b2 += o[0:EWP, 42].sum()
        s_cos += o[0:EWP, 43].sum()
        s_unc += o[0:EWP, 44].sum()
        if c % 4 == 0:
            s_C += o[:, 45].sum()
        s_A += o[:, 46:54].sum()
        s_m1 += np.maximum(o[:, 54:62], 0.0).sum()
    s_m2 = sum(np.maximum(m2[b], 0.0).sum() for b in range(B))

    sdf_loss = s_l1 / (B * P)
    bce = -(s_b1 + s_b2) / (B * P)
    p_t = np.exp(-bce)
    occ_loss = FOCAL_ALPHA * (1.0 - p_t) ** FOCAL_GAMMA * bce
    normal_loss = 1.0 - s_cos / (B * P)
    unc_reg = s_unc / (B * P)
    chamfer = s_m1 / (B * N) + s_m2 / (B * M)
    emd = (s_A / N + s_C / M) / B
    gp = s_gp / (B * N)
    total = (W_SDF * sdf_loss + W_OCC * occ_loss + W_NORMAL * normal_loss
             + W_CHAMFER * chamfer + W_EMD * emd + W_EMD_GP * gp
             + W_UNC * unc_reg)
    return np.float32(total)


def run(inputs, trace=False):
    nc = _get_nc()
    in_maps = _pack_core_inputs(inputs)
    res = run_bass_kernel_spmd(nc, in_maps, list(range(8)), trace=trace)
    total = _combine(res.results)
    return total, res


def kernel(**inputs) -> np.ndarray:
    total, _ = run(inputs)
    return np.asarray(total, dtype=np.float32)
